# revision 1
# baseline (speedup 1.0000x reference)
"""Trainium2 Bass kernel for nn_NewModel_42356967473589 (dense_transformer).

Model: two BiAttention blocks + final linear mapping.
  o = BiAttn(ctx, q1) ; o = BiAttn(o, q2) ; out = o @ w_map.T + b_map

Sharding: 8 cores = (batch b in 0..3) x (context half h in 0..1).
Each core owns 1024 context rows of one batch. All compute is row-local
except the softmax-over-context (weight_two); its (sum-exp, weighted-sum)
stats are combined across the pair of cores sharing a batch via a tiny
pairwise AllReduce, overlapped with the large matmuls.

Math restructure (per stage, X = stage input [C,D], M = memory [Q,D]):
  out = X@W1 + o1@W2 + (X*o1)@W3 + (t*o1)@W4      (W_k = w_out[:, kD:(k+1)D].T)
  o1 = P@M (rank Q=64), t broadcast over rows =>
  o1@W2 + (t*o1)@W4 = P @ (M @ (W2 + t*W4))        (rank-64 path)
All tensors are kept transposed on-chip ([D on partitions, rows free]) so
every matmul runs with a >=256 moving dim at full fp32r (TF32-like) rate,
and each stage emits its output already transposed for the next stage.
"""

import numpy as np

import concourse.bacc as bacc
import concourse.tile as tile
from concourse import mybir
from concourse.bass_utils import run_bass_kernel_spmd
from contextlib import ExitStack
import bass_rust

f32 = mybir.dt.float32
f32r = mybir.dt.float32r
i32 = mybir.dt.int32
Alu = mybir.AluOpType
AF = bass_rust.ActivationFunctionType
AX = bass_rust.AxisListType
RedOp = bass_rust.ReduceOp

B, C_LEN, Q_LEN, D = 4, 2048, 64, 1024
N_CORES = 8
R = C_LEN // 2          # rows per core
NK = D // 128           # contraction chunks
RH = R // 512           # row halves (moving-dim tiles)
D2 = 2 * D
NEGBIG = 10000.0

_CACHED_NC = None


def _build_nc():
    nc = bacc.Bacc("TRN2", target_bir_lowering=False, debug=False,
                   num_devices=N_CORES)

    # ---- per-core DRAM I/O (host pre-tiled layouts, see _shard_inputs) ----
    xt_ap = nc.dram_tensor("xt", [128, NK * R], f32r, kind="ExternalInput").ap()
    m_t = [nc.dram_tensor(f"m{s}t", [128, NK * Q_LEN], f32r, kind="ExternalInput").ap() for s in (1, 2)]
    m_n = [nc.dram_tensor(f"m{s}n", [Q_LEN, D], f32r, kind="ExternalInput").ap() for s in (1, 2)]
    vec = [nc.dram_tensor(f"vec{s}", [128, NK * 3], f32r, kind="ExternalInput").ap() for s in (1, 2)]
    msk = [nc.dram_tensor(f"mask{s}", [Q_LEN, 1], i32, kind="ExternalInput").ap() for s in (1, 2)]
    w1t = [nc.dram_tensor(f"w1t{s}", [D, D], f32r, kind="ExternalInput").ap() for s in (1, 2)]
    w3t = [nc.dram_tensor(f"w3t{s}", [D, D], f32r, kind="ExternalInput").ap() for s in (1, 2)]
    w2c = [nc.dram_tensor(f"w2c{s}", [D, D], f32r, kind="ExternalInput").ap() for s in (1, 2)]
    w4c = [nc.dram_tensor(f"w4c{s}", [D, D], f32r, kind="ExternalInput").ap() for s in (1, 2)]
    wmt_ap = nc.dram_tensor("wmt", [D2, D], f32r, kind="ExternalInput").ap()
    bmap_ap = nc.dram_tensor("bmap", [D2, 1], f32r, kind="ExternalInput").ap()
    out_ap = nc.dram_tensor("out", [D2, R], f32, kind="ExternalOutput").ap()

    with tile.TileContext(nc) as tc, ExitStack() as ctx:
        sb_xt = ctx.enter_context(tc.tile_pool(name="sb_xt", bufs=2))
        sb_xo = ctx.enter_context(tc.tile_pool(name="sb_xo", bufs=1))
        sb_w13 = ctx.enter_context(tc.tile_pool(name="sb_w13", bufs=6))
        sb_ws = ctx.enter_context(tc.tile_pool(name="sb_ws", bufs=2))
        sb_rh = ctx.enter_context(tc.tile_pool(name="sb_rh", bufs=2))
        sb_st = ctx.enter_context(tc.tile_pool(name="sb_st", bufs=1))
        ps_o = ctx.enter_context(tc.tile_pool(name="ps_o", bufs=2, space="PSUM"))
        ps_att = ctx.enter_context(tc.tile_pool(name="ps_att", bufs=2, space="PSUM"))
        ps_rr = ctx.enter_context(tc.tile_pool(name="ps_rr", bufs=2, space="PSUM"))
        ps_m = ctx.enter_context(tc.tile_pool(name="ps_m", bufs=2, space="PSUM"))
        dram = ctx.enter_context(tc.tile_pool(name="dram", bufs=2, space="DRAM"))

        # ---- constants ----
        ones_row = sb_st.tile([1, 128], f32r, tag="ones_row")
        nc.vector.memset(ones_row[:].bitcast(f32), 1.0)
        ones_q = sb_st.tile([Q_LEN, 1], f32r, tag="ones_q")
        nc.vector.memset(ones_q[:].bitcast(f32), 1.0)

        # ---- stage input 1: X^T ----
        xt0 = sb_xt.tile([128, NK, R], f32r, tag="xt")
        for c in range(NK):
            nc.gpsimd.dma_start(xt0[:, c], xt_ap[:, c * R:(c + 1) * R])

        def run_stage(s, Xt):
            """One BiAttention stage; returns o^T tile [128, NK, R] f32r."""
            sfx = f"_s{s}"
            # ---------- stage constants ----------
            vecs = sb_st.tile([128, NK, 3], f32r, tag="vecs" + sfx)
            nc.sync.dma_start(vecs[:], vec[s][:].rearrange("p (c k) -> p c k", c=NK))
            mT = sb_st.tile([128, NK, Q_LEN], f32r, tag="mT" + sfx)
            nc.sync.dma_start(mT[:], m_t[s][:].rearrange("p (c q) -> p c q", c=NK))
            mN = sb_st.tile([Q_LEN, D], f32r, tag="mN" + sfx)
            nc.sync.dma_start(mN[:], m_n[s][:])
            mask_i = sb_st.tile([Q_LEN, 1], i32, tag="mask_i" + sfx)
            nc.sync.dma_start(mask_i[:], msk[s][:])

            # memory_dot = M @ w_mem  -> psum [Q,1]
            ps_md = ps_m.tile([Q_LEN, 2], f32, tag="ps_m")
            for c in range(NK):
                nc.tensor.matmul(ps_md[:], mT[:, c], vecs[:, c, 1:3],
                                 start=(c == 0), stop=(c == NK - 1))
            # mbias = memory_dot + (mask-1)*NEGBIG
            maskf = sb_st.tile([Q_LEN, 1], f32, tag="maskf" + sfx)
            nc.vector.tensor_copy(maskf[:], mask_i[:])
            mbias = sb_st.tile([Q_LEN, 1], f32, tag="mbias" + sfx)
            nc.vector.tensor_scalar(mbias[:], maskf[:], NEGBIG, -NEGBIG, Alu.mult, Alu.add)
            nc.vector.tensor_tensor(mbias[:], mbias[:], ps_md[:, 0:1], Alu.add)

            # mst = [M^T * scale | w_in]  (lhsT for the score matmul)
            mst = sb_st.tile([128, NK, Q_LEN + 1], f32r, tag="mst" + sfx)
            nc.vector.tensor_copy(mst[:, :, 0:Q_LEN], mT[:])
            nc.vector.tensor_copy(mst[:, :, Q_LEN:Q_LEN + 1], vecs[:, :, 0:1])
            for c in range(NK):
                nc.vector.tensor_scalar(mst[:, c, 0:Q_LEN], mst[:, c, 0:Q_LEN],
                                        vecs[:, c, 2:3].bitcast(f32), None, Alu.mult)

            P = sb_st.tile([Q_LEN, R], f32r, tag="P" + sfx)
            vh = sb_st.tile([128, 2 * NK], f32, tag="vh" + sfx)
            l2col = sb_st.tile([1, 2], f32, tag="l2col" + sfx)

            for rh in range(RH):
                sl = slice(rh * 512, (rh + 1) * 512)
                # scores S' = mst.T @ X^T -> [Q+1, 512] in psum
                ps_sc = ps_att.tile([Q_LEN + 1, 512], f32, tag="ps_sc")
                for c in range(NK):
                    nc.tensor.matmul(ps_sc[:], mst[:, c], Xt[:, c, sl],
                                     start=(c == 0), stop=(c == NK - 1))
                # E = exp(S + membias) (masked -> 0); eid = exp(input_dot)
                E = sb_rh.tile([Q_LEN, 512], f32r, tag="E")
                eid = sb_rh.tile([1, 512], f32, tag="eid")
                nc.scalar.activation(E[:], ps_sc[0:Q_LEN], AF.Exp,
                                     bias=mbias[:], scale=1.0)
                nc.scalar.activation(eid[:], ps_sc[Q_LEN:Q_LEN + 1], AF.Exp)
                # l1 = column sums of E; rl1 = 1/l1
                ps_l1 = ps_m.tile([1, 512], f32, tag="ps_m")
                nc.tensor.matmul(ps_l1[:], ones_q[:], E[:], start=True, stop=True)
                l1r = sb_rh.tile([1, 512], f32r, tag="l1r")
                with nc.allow_low_precision(reason="softmax scale in f32r"):
                    nc.vector.reciprocal(l1r[:], ps_l1[:])
                # P = E * (1/l1) broadcast over partitions
                ps_rb = ps_m.tile([Q_LEN, 512], f32, tag="ps_m")
                nc.tensor.matmul(ps_rb[:], ones_row[:, 0:Q_LEN], l1r[:],
                                 start=True, stop=True)
                nc.vector.tensor_tensor(P[:, sl], E[:].bitcast(f32), ps_rb[:], Alu.mult)

                # weight_two stats: m_exp = max_q E; e2 = m_exp * exp(input_dot)
                mx = sb_rh.tile([Q_LEN, 512], f32, tag="mx")
                nc.gpsimd.partition_all_reduce(mx[:], E[:].bitcast(f32), Q_LEN,
                                               RedOp.max)
                e2 = sb_rh.tile([1, 512], f32r, tag="e2")
                nc.vector.tensor_tensor(e2[:], mx[0:1], eid[:], Alu.mult)
                nc.vector.reduce_sum(l2col[:, rh:rh + 1], e2[:].bitcast(f32), AX.X)
                # v partial sums: vh[:, 2c+rh] = sum_sl e2 * Xt[:, c, sl]
                ps_eb = ps_m.tile([128, 512], f32, tag="ps_m")
                nc.tensor.matmul(ps_eb[:], ones_row[:], e2[:], start=True, stop=True)
                scrv = sb_rh.tile([128, 512], f32, tag="scrv")
                for c in range(NK):
                    nc.vector.scalar_tensor_tensor(
                        scrv[:], Xt[:, c, sl].bitcast(f32), 1.0, ps_eb[:],
                        Alu.mult, Alu.mult,
                        accum_out=vh[:, 2 * c + rh:2 * c + rh + 1])

            l2 = sb_st.tile([1, 1], f32, tag="l2" + sfx)
            nc.vector.reduce_sum(l2[:], l2col[:], AX.X)
            vsum = sb_st.tile([128, NK], f32, tag="vsum" + sfx)
            vh3 = vh[:].rearrange("p (c t) -> p c t", t=2)
            nc.vector.tensor_tensor(vsum[:], vh3[:, :, 0], vh3[:, :, 1], Alu.add)

            # ---------- pairwise AllReduce of (v, l2) ----------
            colsb = sb_st.tile([128, 16], f32, tag="colsb" + sfx)
            nc.vector.memset(colsb[:], 0.0)
            nc.vector.tensor_copy(colsb[:, 0:NK], vsum[:])
            nc.vector.tensor_copy(colsb[0:1, NK:NK + 1], l2[:])
            nc.vector.tensor_copy(colsb[0:1, NK + 1:NK + 2], l2[:])
            cin = dram.tile([128, 16], f32, tag="cin" + sfx)
            cout = dram.tile([128, 16], f32, tag="cout" + sfx)
            nc.sync.dma_start(cin[:], colsb[:])
            nc.gpsimd.collective_compute(
                "AllReduce", Alu.add,
                replica_groups=[[0, 1], [2, 3], [4, 5], [6, 7]],
                ins=[cin[:].opt()], outs=[cout[:].opt()])
            colg = sb_st.tile([128, 16], f32, tag="colg" + sfx)
            nc.sync.dma_start(colg[:], cout[:])

            # W2/W4 streamed on the ACT hwdge queue (starts early, consumed late)
            w2t_ch = []
            w4t_ch = []
            for c in range(NK):
                w2h = sb_ws.tile([128, 1024], f32r, tag="w2h")
                nc.scalar.dma_start(w2h[:], w2c[s][c * 128:(c + 1) * 128, :])
                w2t_ch.append(w2h)
            for c in range(NK):
                w4h = sb_ws.tile([128, 1024], f32r, tag="w4h")
                nc.scalar.dma_start(w4h[:], w4c[s][c * 128:(c + 1) * 128, :])
                w4t_ch.append(w4h)

            oT = sb_xt.tile([128, NK, R], f32r, tag="xt")
            # pre-open the j=0 A-groups so PE has work while DVE does v/XO
            w13_tiles = {}
            def load_w13(j):
                w1j = sb_w13.tile([128, NK, 128], f32r, tag="w13")
                w3j = sb_w13.tile([128, NK, 128], f32r, tag="w13")
                nc.sync.dma_start(w1j[:], w1t[s][j * 128:(j + 1) * 128, :]
                                  .rearrange("p (c m) -> p c m", c=NK))
                nc.sync.dma_start(w3j[:], w3t[s][j * 128:(j + 1) * 128, :]
                                  .rearrange("p (c m) -> p c m", c=NK))
                w13_tiles[j] = (w1j, w3j)

            load_w13(0)
            pre_groups = []
            for rh in range(RH):
                sl = slice(rh * 512, (rh + 1) * 512)
                ps_ab = ps_o.tile([128, 512], f32, tag="ps_o")
                for c in range(NK):
                    nc.tensor.matmul(ps_ab[:], w13_tiles[0][0][:, c], Xt[:, c, sl],
                                     start=(c == 0), stop=False)
                pre_groups.append((rh, ps_ab))

            # ---------- o1^T = mN.T @ P and XO = Xt * o1 ----------
            XO = sb_xo.tile([128, NK, R], f32r, tag="xo")
            for c in range(NK):
                for rh in range(RH):
                    sl = slice(rh * 512, (rh + 1) * 512)
                    ps_o1 = ps_att.tile([128, 512], f32, tag="ps_sc")
                    nc.tensor.matmul(ps_o1[:], mN[:, c * 128:(c + 1) * 128], P[:, sl],
                                     start=True, stop=True)
                    nc.vector.tensor_tensor(XO[:, c, sl],
                                            Xt[:, c, sl].bitcast(f32), ps_o1[:], Alu.mult)

            ps_r = []
            for i in range(2):
                ps_ri = ps_rr.tile([Q_LEN, 512], f32, tag="ps_r")
                ps_r.append(ps_ri)

            # close the pre-opened j=0 groups with their W3 halves
            for rh, ps_ab in pre_groups:
                sl = slice(rh * 512, (rh + 1) * 512)
                for c in range(NK):
                    nc.tensor.matmul(ps_ab[:], w13_tiles[0][1][:, c], XO[:, c, sl],
                                     start=False, stop=(c == NK - 1))
                nc.vector.tensor_copy(oT[:, 0, sl], ps_ab[:])

            # ---------- remaining big blocks ----------
            for j in range(1, NK):
                load_w13(j)
                w1j, w3j = w13_tiles[j]
                for rh in range(RH):
                    sl = slice(rh * 512, (rh + 1) * 512)
                    ps_ab = ps_o.tile([128, 512], f32, tag="ps_o")
                    for c in range(NK):
                        nc.tensor.matmul(ps_ab[:], w1j[:, c], Xt[:, c, sl],
                                         start=(c == 0), stop=False)
                    for c in range(NK):
                        nc.tensor.matmul(ps_ab[:], w3j[:, c], XO[:, c, sl],
                                         start=False, stop=(c == NK - 1))
                    nc.vector.tensor_copy(oT[:, j, sl], ps_ab[:])
                if j == 1:
                    for c in range(NK):
                        for hf in range(2):
                            slh = slice(hf * 512, (hf + 1) * 512)
                            nc.tensor.matmul(ps_r[hf][:], mT[:, c],
                                             w2t_ch[c][:, slh],
                                             start=(c == 0), stop=False)

            # ---------- collective-dependent tail ----------
            linv = sb_st.tile([1, 2], f32r, tag="linv" + sfx)
            with nc.allow_low_precision(reason="weight-two scale in f32r"):
                nc.vector.reciprocal(linv[:], colg[0:1, NK:NK + 2])
            ps_tb = ps_m.tile([128, 2], f32, tag="ps_m")
            nc.tensor.matmul(ps_tb[:], ones_row[:], linv[:], start=True, stop=True)
            tvec = sb_st.tile([128, NK], f32, tag="tvec" + sfx)
            nc.vector.tensor_scalar(tvec[:], colg[:, 0:NK], ps_tb[:, 0:1], None, Alu.mult)

            for c in range(NK):
                w24 = sb_ws.tile([128, 1024], f32r, tag="w24")
                nc.vector.tensor_scalar(w24[:], w4t_ch[c][:].bitcast(f32),
                                        tvec[:, c:c + 1], None, Alu.mult)
                for hf in range(2):
                    sl = slice(hf * 512, (hf + 1) * 512)
                    nc.tensor.matmul(ps_r[hf][:], mT[:, c], w24[:, sl],
                                     start=False, stop=(c == NK - 1))
            Rsb = sb_st.tile([Q_LEN, D], f32r, tag="Rsb" + sfx)
            for hf in range(2):
                nc.vector.tensor_copy(Rsb[:, hf * 512:(hf + 1) * 512], ps_r[hf][:])

            # rank-64 correction (rh-major so the next stage starts sooner)
            for rh in range(RH):
                for j in range(NK):
                    sl = slice(rh * 512, (rh + 1) * 512)
                    ps_c = ps_att.tile([128, 512], f32, tag="ps_sc")
                    nc.tensor.matmul(ps_c[:], Rsb[:, j * 128:(j + 1) * 128], P[:, sl],
                                     start=True, stop=True)
                    nc.vector.tensor_tensor(oT[:, j, sl],
                                            oT[:, j, sl].bitcast(f32), ps_c[:], Alu.add)
            return oT

        o1T = run_stage(0, xt0)
        o2T = run_stage(1, o1T)

        # ---------- final linear (transposed): outT = w_mapT.T @ o2T + b ----
        for j2 in range(16):
            wmj = sb_w13.tile([128, NK, 128], f32r, tag="w13")
            nc.scalar.dma_start(wmj[:], wmt_ap[j2 * 128:(j2 + 1) * 128, :]
                                .rearrange("p (c m) -> p c m", c=NK))
            bcol = sb_ws.tile([128, 1], f32, tag="bmj")
            nc.scalar.dma_start(bcol[:], bmap_ap[j2 * 128:(j2 + 1) * 128, :].bitcast(f32))
            for rh in range(RH):
                sl = slice(rh * 512, (rh + 1) * 512)
                ps_f = ps_o.tile([128, 512], f32, tag="ps_o")
                for c in range(NK):
                    nc.tensor.matmul(ps_f[:], wmj[:, c], o2T[:, c, sl],
                                     start=(c == 0), stop=(c == NK - 1))
                outsb = sb_ws.tile([128, 512], f32, tag="outsb")
                nc.vector.tensor_scalar(outsb[:], ps_f[:], bcol[:], None, Alu.add)
                nc.sync.dma_start(
                    out_ap[j2 * 128:(j2 + 1) * 128, sl], outsb[:])

    nc.compile()
    return nc


def _get_nc():
    global _CACHED_NC
    if _CACHED_NC is None:
        _CACHED_NC = _build_nc()
    return _CACHED_NC


def _shard_inputs(inputs):
    """Build the 8 per-core input maps (pure layout work, no arithmetic)."""
    x = np.ascontiguousarray(inputs["ctx_features"], dtype=np.float32)
    q1 = np.ascontiguousarray(inputs["sub_q1_features"], dtype=np.float32)
    q2 = np.ascontiguousarray(inputs["sub_q2_features"], dtype=np.float32)
    k1 = np.ascontiguousarray(inputs["sub_q1_attn_mask"], dtype=np.int32)
    k2 = np.ascontiguousarray(inputs["sub_q2_attn_mask"], dtype=np.int32)

    def wblocks(w_out):
        # w_out [D, 4D] -> wb = w_out.T [4D, D]; W_k = wb[kD:(k+1)D]
        wb = np.ascontiguousarray(w_out.T, dtype=np.float32)
        W1, W2, W3, W4 = (wb[k * D:(k + 1) * D] for k in range(4))

        def jmaj(W):  # j-major tiling for the lhsT stream
            return np.ascontiguousarray(
                W.reshape(NK, 128, NK, 128).transpose(2, 1, 0, 3).reshape(D, D))
        return jmaj(W1), np.ascontiguousarray(W2), jmaj(W3), np.ascontiguousarray(W4)

    w1t1, w2c1, w3t1, w4c1 = wblocks(inputs["w_out1"])
    w1t2, w2c2, w3t2, w4c2 = wblocks(inputs["w_out2"])

    wmT = inputs["w_map"].T.astype(np.float32)  # [D, 2D]
    wmt = np.ascontiguousarray(
        wmT.reshape(NK, 128, 16, 128).transpose(2, 1, 0, 3).reshape(D2, D))
    bmap = np.ascontiguousarray(
        np.asarray(inputs["b_map"], dtype=np.float32).reshape(D2, 1))

    def ptile_vec(*cols):  # [D] vectors -> [128, NK*k] p-major
        v = np.stack([np.asarray(c, dtype=np.float32) for c in cols], axis=-1)
        k = v.shape[-1]
        return np.ascontiguousarray(
            v.reshape(NK, 128, k).transpose(1, 0, 2).reshape(128, NK * k))

    stage_common = {
        "vec1": ptile_vec(inputs["w_in1"], inputs["w_mem1"], inputs["scale1"]),
        "vec2": ptile_vec(inputs["w_in2"], inputs["w_mem2"], inputs["scale2"]),
        "w1t1": w1t1, "w3t1": w3t1, "w2c1": w2c1, "w4c1": w4c1,
        "w1t2": w1t2, "w3t2": w3t2, "w2c2": w2c2, "w4c2": w4c2,
        "wmt": wmt, "bmap": bmap,
    }

    in_maps = []
    for core in range(N_CORES):
        b, h = divmod(core, 2)
        xT = x[b, h * R:(h + 1) * R, :].T  # [D, R]
        xt_tile = np.ascontiguousarray(
            xT.reshape(NK, 128, R).transpose(1, 0, 2).reshape(128, NK * R))
        m = {}
        for s, q, kk in ((1, q1, k1), (2, q2, k2)):
            mT = q[b].T  # [D, Q]
            m[f"m{s}t"] = np.ascontiguousarray(
                mT.reshape(NK, 128, Q_LEN).transpose(1, 0, 2).reshape(128, NK * Q_LEN))
            m[f"m{s}n"] = np.ascontiguousarray(q[b])
            m[f"mask{s}"] = np.ascontiguousarray(kk[b].reshape(Q_LEN, 1))
        in_maps.append({"xt": xt_tile, **m, **stage_common})
    return in_maps


def _gather_outputs(results):
    out = np.empty((B, C_LEN, D2), dtype=np.float32)
    for core in range(N_CORES):
        b, h = divmod(core, 2)
        out[b, h * R:(h + 1) * R, :] = results[core]["out"].T
    return out


def kernel(**inputs):
    nc = _get_nc()
    in_maps = _shard_inputs(inputs)
    last_err = None
    for _attempt in range(3):
        try:
            res = run_bass_kernel_spmd(nc, in_maps, core_ids=list(range(N_CORES)))
            return _gather_outputs(res.results)
        except Exception as e:  # transient device errors: retry
            last_err = e
    raise last_err



# revision 9
# speedup vs baseline: 1.1761x; 1.1761x over previous
"""Trainium2 Bass kernel for nn_NewModel_42356967473589 (dense_transformer).

Model: two BiAttention blocks + final linear mapping.
  o = BiAttn(ctx, q1) ; o = BiAttn(o, q2) ; out = o @ w_map.T + b_map

Sharding: 8 cores = (batch b in 0..3) x (context half h in 0..1).
Each core owns 1024 context rows of one batch. All compute is row-local
except the softmax-over-context (weight_two); its (sum-exp, weighted-sum)
stats are combined across the pair of cores sharing a batch via a tiny
pairwise AllReduce, overlapped with the large matmuls.

Math restructure (per stage, X = stage input [C,D], M = memory [Q,D]):
  out = X@W1 + o1@W2 + (X*o1)@W3 + (t*o1)@W4      (W_k = w_out[:, kD:(k+1)D].T)
  o1 = P@M (rank Q=64), t broadcast over rows =>
  o1@W2 + (t*o1)@W4 = P @ (M@W2 + (M*t)@W4)        (rank-64 path)

All big operands are bf16 (matmul rate on TRN2 is the same as f32r at
>=256 moving rows, but DMA/SBUF/DVE traffic halves); accumulation stays in
fp32 PSUM and softmax statistics are computed in fp32. The column-softmax
max/sum stats are obtained by PE-transposing the exp'd score tile and
reducing along the free axis on DVE (the gpsimd partition_all_reduce the
earlier version used was ~16.5us per call and serialized the stage).
"""

import numpy as np
import ml_dtypes

import concourse.bacc as bacc
import concourse.tile as tile
from concourse import mybir
from concourse.bass_utils import run_bass_kernel_spmd
from concourse.masks import make_identity
from contextlib import ExitStack
import bass_rust

f32 = mybir.dt.float32
bf16 = mybir.dt.bfloat16
i32 = mybir.dt.int32
Alu = mybir.AluOpType
AF = bass_rust.ActivationFunctionType
AX = bass_rust.AxisListType

B, C_LEN, Q_LEN, D = 4, 2048, 64, 1024
N_CORES = 8
R = C_LEN // 2          # rows per core
NK = D // 128           # contraction chunks
RH = R // 512           # row halves (moving-dim tiles)
NJ2 = 16                # output blocks of the final linear
D2 = 2 * D
NEGBIG = 10000.0
BF = ml_dtypes.bfloat16

_CACHED_NC = None


def _build_nc():
    nc = bacc.Bacc("TRN2", target_bir_lowering=False, debug=False,
                   num_devices=N_CORES)

    # ---- per-core DRAM I/O (host pre-tiled layouts, see _shard_inputs) ----
    xt_ap = nc.dram_tensor("xt", [128, NK * R], bf16, kind="ExternalInput").ap()
    m_t = [nc.dram_tensor(f"m{s}t", [128, NK * Q_LEN], bf16, kind="ExternalInput").ap() for s in (1, 2)]
    m_n = [nc.dram_tensor(f"m{s}n", [Q_LEN, D], bf16, kind="ExternalInput").ap() for s in (1, 2)]
    vec = [nc.dram_tensor(f"vec{s}", [128, NK * 3], f32, kind="ExternalInput").ap() for s in (1, 2)]
    wmb = [nc.dram_tensor(f"wmb{s}", [128, NK], bf16, kind="ExternalInput").ap() for s in (1, 2)]
    msk = [nc.dram_tensor(f"mask{s}", [Q_LEN, 1], i32, kind="ExternalInput").ap() for s in (1, 2)]
    w1t = [nc.dram_tensor(f"w1t{s}", [D, D], bf16, kind="ExternalInput").ap() for s in (1, 2)]
    w3t = [nc.dram_tensor(f"w3t{s}", [D, D], bf16, kind="ExternalInput").ap() for s in (1, 2)]
    w2c = [nc.dram_tensor(f"w2c{s}", [D, D], bf16, kind="ExternalInput").ap() for s in (1, 2)]
    w4c = [nc.dram_tensor(f"w4c{s}", [D, D], bf16, kind="ExternalInput").ap() for s in (1, 2)]
    wmt_ap = nc.dram_tensor("wmt", [D2, D], bf16, kind="ExternalInput").ap()
    bmap_ap = nc.dram_tensor("bmap", [128, NJ2], f32, kind="ExternalInput").ap()
    out_ap = nc.dram_tensor("out", [D2, R], bf16, kind="ExternalOutput").ap()

    with tile.TileContext(nc) as tc, ExitStack() as ctx:
        sb = ctx.enter_context(tc.tile_pool(name="sb", bufs=1))
        sb_xt = ctx.enter_context(tc.tile_pool(name="sb_xt", bufs=2))
        sb_xo = ctx.enter_context(tc.tile_pool(name="sb_xo", bufs=1))
        sb_w13 = ctx.enter_context(tc.tile_pool(name="sb_w13", bufs=6))
        sb_wmj = ctx.enter_context(tc.tile_pool(name="sb_wmj", bufs=16))
        sb_w24 = ctx.enter_context(tc.tile_pool(name="sb_w24", bufs=8))
        ps_sc = ctx.enter_context(tc.tile_pool(name="ps_sc", bufs=2, space="PSUM"))
        ps_tp = ctx.enter_context(tc.tile_pool(name="ps_tp", bufs=2, space="PSUM"))
        ps_bc = ctx.enter_context(tc.tile_pool(name="ps_bc", bufs=2, space="PSUM"))
        ps_jg = ctx.enter_context(tc.tile_pool(name="ps_jg", bufs=2, space="PSUM"))
        dram = ctx.enter_context(tc.tile_pool(name="dram", bufs=2, space="DRAM"))

        # ---- constants ----
        ident = sb.tile([128, 128], bf16, tag="ident")
        make_identity(nc, ident[:])
        ones_r = sb.tile([1, 128], bf16, tag="ones_r")
        nc.vector.memset(ones_r[:], 1.0)
        ones_c = sb.tile([128, 1], bf16, tag="ones_c")
        nc.vector.memset(ones_c[:], 1.0)

        # ---- stage input 1: X^T (software-DGE queue) ----
        xt0 = sb_xt.tile([128, NK, R], bf16, tag="xt")
        for c in range(NK):
            nc.gpsimd.dma_start(xt0[:, c], xt_ap[:, c * R:(c + 1) * R])

        # ---- per-stage constants + memory_dot / mbias, both stages up front
        st = [dict() for _ in range(2)]

        def prep_stage(s):
            d = st[s]
            vecs = sb.tile([128, NK, 3], f32, tag=f"vecs{s}")
            nc.sync.dma_start(vecs[:], vec[s][:].rearrange("p (c k) -> p c k", c=NK))
            mT = sb.tile([128, NK, Q_LEN], bf16, tag=f"mT{s}")
            nc.sync.dma_start(mT[:], m_t[s][:].rearrange("p (c q) -> p c q", c=NK))
            wmbs = sb.tile([128, NK], bf16, tag=f"wmb{s}")
            nc.sync.dma_start(wmbs[:], wmb[s][:])
            mN = sb.tile([Q_LEN, D], bf16, tag=f"mN{s}")
            nc.sync.dma_start(mN[:], m_n[s][:])
            mask_i = sb.tile([Q_LEN, 1], i32, tag=f"mask{s}")
            nc.sync.dma_start(mask_i[:], msk[s][:])

            # mst = [M^T * scale | w_in]  (lhsT for the score matmul)
            mst = sb.tile([128, NK, Q_LEN + 1], bf16, tag=f"mst{s}")
            for c in range(NK):
                nc.vector.tensor_scalar(mst[:, c, 0:Q_LEN], mT[:, c],
                                        vecs[:, c, 2:3], None, Alu.mult)
            nc.vector.tensor_copy(mst[:, :, Q_LEN:Q_LEN + 1], vecs[:, :, 0:1])

            # memory_dot = M @ w_mem  -> psum [Q,1]
            ps_md = ps_tp.tile([Q_LEN, 1], f32, tag="tp")
            for c in range(NK):
                nc.tensor.matmul(ps_md[:], mT[:, c], wmbs[:, c:c + 1],
                                 start=(c == 0), stop=(c == NK - 1))
            maskf = sb.tile([Q_LEN, 1], f32, tag=f"maskf{s}")
            nc.vector.tensor_copy(maskf[:], mask_i[:])
            mbias = sb.tile([Q_LEN, 1], f32, tag=f"mbias{s}")
            nc.vector.tensor_scalar(mbias[:], maskf[:], NEGBIG, -NEGBIG,
                                    Alu.mult, Alu.add)
            nc.vector.tensor_tensor(mbias[:], mbias[:], ps_md[:], Alu.add)
            d.update(mT=mT, mN=mN, mst=mst, mbias=mbias)

        def load_w24(s):
            w2ch, w4ch = [], []
            for c in range(NK):
                w2h = sb_w24.tile([128, 1024], bf16, tag="w2h")
                nc.scalar.dma_start(w2h[:], w2c[s][c * 128:(c + 1) * 128, :])
                w2ch.append(w2h)
            for c in range(NK):
                w4h = sb_w24.tile([128, 1024], bf16, tag="w4h")
                nc.scalar.dma_start(w4h[:], w4c[s][c * 128:(c + 1) * 128, :])
                w4ch.append(w4h)
            st[s].update(w2ch=w2ch, w4ch=w4ch)

        prep_stage(0)
        prep_stage(1)
        load_w24(0)
        load_w24(1)

        # final-linear weights: all 16 blocks resident (scalar hwdge queue)
        wmjs = []
        for j2 in range(NJ2):
            wmj = sb_wmj.tile([128, NK, 128], bf16, tag="wmj")
            nc.scalar.dma_start(wmj[:], wmt_ap[j2 * 128:(j2 + 1) * 128, :]
                                .rearrange("p (c m) -> p c m", c=NK))
            wmjs.append(wmj)
        bcols = sb.tile([128, NJ2], f32, tag="bcols")
        nc.scalar.dma_start(bcols[:], bmap_ap[:])

        def run_stage(s, Xt):
            """One BiAttention stage; returns o^T tile [128, NK, R] bf16."""
            d = st[s]
            mT, mN, mst, mbias = d["mT"], d["mN"], d["mst"], d["mbias"]
            w2ch, w4ch = d["w2ch"], d["w4ch"]

            w13 = {}

            def load_w13(j):
                w1j = sb_w13.tile([128, NK, 128], bf16, tag="w13")
                nc.sync.dma_start(w1j[:], w1t[s][j * 128:(j + 1) * 128, :]
                                  .rearrange("p (c m) -> p c m", c=NK))
                w3j = sb_w13.tile([128, NK, 128], bf16, tag="w13")
                nc.sync.dma_start(w3j[:], w3t[s][j * 128:(j + 1) * 128, :]
                                  .rearrange("p (c m) -> p c m", c=NK))
                w13[j] = (w1j, w3j)

            load_w13(0)

            P = sb.tile([Q_LEN, R], bf16, tag="P", bufs=2)
            st8 = sb.tile([128, RH, 8], bf16, tag="st8", bufs=2)
            vh = sb.tile([128, 2 * NK], f32, tag="vh", bufs=2)
            E_ = [None] * RH
            TP_ = [None] * RH
            rs_ = [None] * RH
            ebs_ = [None] * RH

            def scores(rh):
                sl = slice(rh * 512, (rh + 1) * 512)
                ps = ps_sc.tile([Q_LEN + 1, 512], f32, tag="sc")
                for c in range(NK):
                    nc.tensor.matmul(ps[:], mst[:, c], Xt[:, c, sl],
                                     start=(c == 0), stop=(c == NK - 1))
                E = sb.tile([Q_LEN + 1, 512], bf16, tag="E", bufs=2)
                nc.scalar.activation(E[0:Q_LEN], ps[0:Q_LEN], AF.Exp,
                                     bias=mbias[:], scale=1.0)
                nc.scalar.activation(E[Q_LEN:Q_LEN + 1], ps[Q_LEN:Q_LEN + 1],
                                     AF.Exp)
                E_[rh] = E

            def transposes(rh):
                # E chunks [65,128] -> [128,65] so q lands on the free axis
                TP = ps_tp.tile([128, 4, 80], bf16, tag="tp")
                for k in range(4):
                    nc.tensor.transpose(TP[:, k, 0:Q_LEN + 1],
                                        E_[rh][:, k * 128:(k + 1) * 128],
                                        ident[0:Q_LEN + 1, 0:Q_LEN + 1])
                TP_[rh] = TP

            def stats(rh):
                TP = TP_[rh]
                mx = sb.tile([128, 4], f32, tag="mx", bufs=2)
                nc.vector.reduce_max(mx[:], TP[:, :, 0:Q_LEN], AX.X)
                l1 = sb.tile([128, 4], f32, tag="l1", bufs=2)
                nc.vector.reduce_sum(l1[:], TP[:, :, 0:Q_LEN], AX.X)
                # e2 = (max_q E) * exp(input_dot); l1r = 1/l1
                nc.vector.tensor_tensor(st8[:, rh, 0:4], mx[:],
                                        TP[:, :, Q_LEN], Alu.mult)
                with nc.allow_low_precision(reason="softmax scales in bf16"):
                    nc.vector.reciprocal(st8[:, rh, 4:8], l1[:])

            def stats_rows(rh):
                # column stats [128,{e2|l1r}x4] -> row form [1, 1024] on p0
                tp8 = ps_tp.tile([1, 1024], bf16, tag="tp")
                for k in range(8):
                    nc.tensor.transpose(tp8[0:1, k * 128:(k + 1) * 128],
                                        st8[:, rh, k:k + 1], ident[:])
                rs = sb.tile([1, 1024], bf16, tag="rowst", bufs=2)
                nc.scalar.copy(rs[0:1, 0:512], tp8[0:1, 0:512])
                nc.vector.tensor_copy(rs[0:1, 512:1024], tp8[0:1, 512:1024])
                rs_[rh] = rs

            def bcasts(rh):
                rs = rs_[rh]
                eb = ps_bc.tile([128, 512], f32, tag="bc")
                for k in range(4):
                    nc.tensor.matmul(eb[:, k * 128:(k + 1) * 128], ones_r[:],
                                     rs[0:1, k * 128:(k + 1) * 128],
                                     start=True, stop=True)
                ebs = sb.tile([128, 512], bf16, tag="ebs", bufs=2)
                nc.scalar.copy(ebs[:], eb[:])
                ebs_[rh] = ebs
                rb = ps_bc.tile([Q_LEN, 512], f32, tag="bc")
                for k in range(4):
                    nc.tensor.matmul(rb[:, k * 128:(k + 1) * 128],
                                     ones_r[:, 0:Q_LEN],
                                     rs[0:1, (4 + k) * 128:(5 + k) * 128],
                                     start=True, stop=True)
                sl = slice(rh * 512, (rh + 1) * 512)
                nc.vector.tensor_tensor(P[:, sl], E_[rh][0:Q_LEN], rb[:],
                                        Alu.mult)

            # ---- PE schedule: scores/stats with pre-opened j=0 groups ----
            pre = []
            scores(0)
            for rh in range(RH):
                sl = slice(rh * 512, (rh + 1) * 512)
                pj = ps_jg.tile([128, 512], f32, tag="jg")
                for c in range(NK):
                    nc.tensor.matmul(pj[:], w13[0][0][:, c], Xt[:, c, sl],
                                     start=(c == 0), stop=False)
                pre.append(pj)
                if rh == 0:
                    transposes(0)
                    stats(0)
            scores(1)
            stats_rows(0)
            bcasts(0)
            transposes(1)
            stats(1)
            stats_rows(1)
            bcasts(1)

            # l2 = sum over all rows of e2  (both rh at once)
            ps_l2 = ps_tp.tile([1, RH * 4], f32, tag="tp")
            nc.tensor.matmul(ps_l2[:], ones_c[:], st8[:, :, 0:4],
                             start=True, stop=True)
            l2s = sb.tile([1, 1], f32, tag="l2s", bufs=2)
            nc.vector.reduce_sum(l2s[:], ps_l2[:], AX.X)

            # ---- o1^T = mN.T @ P and XO = Xt * o1 ----
            XO = sb_xo.tile([128, NK, R], bf16, tag="xo")
            for c in range(NK):
                for rh in range(RH):
                    sl = slice(rh * 512, (rh + 1) * 512)
                    ps_o1 = ps_bc.tile([128, 512], f32, tag="bc")
                    nc.tensor.matmul(ps_o1[:], mN[:, c * 128:(c + 1) * 128],
                                     P[:, sl], start=True, stop=True)
                    if c % 2 == 0:
                        nc.vector.tensor_tensor(XO[:, c, sl], Xt[:, c, sl],
                                                ps_o1[:], Alu.mult)
                    else:
                        # spread work: stage PSUM->SBUF via Act, multiply on Pool
                        o1s = sb.tile([128, 512], bf16, tag="o1s", bufs=2)
                        nc.scalar.copy(o1s[:], ps_o1[:])
                        nc.gpsimd.tensor_tensor(XO[:, c, sl], Xt[:, c, sl],
                                                o1s[:], Alu.mult)

            oT = sb_xt.tile([128, NK, R], bf16, tag="xt")
            # close the pre-opened j=0 groups with their W3 halves
            for rh, pj in enumerate(pre):
                sl = slice(rh * 512, (rh + 1) * 512)
                for c in range(NK):
                    nc.tensor.matmul(pj[:], w13[0][1][:, c], XO[:, c, sl],
                                     start=False, stop=(c == NK - 1))
                nc.scalar.copy(oT[:, 0, sl], pj[:])

            # v = sum over rows of e2*X  (accumulated on DVE+Pool from ebs)
            for rh in range(RH):
                sl = slice(rh * 512, (rh + 1) * 512)
                for c in range(NK):
                    scr = sb.tile([128, 512], bf16, tag="scrV", bufs=2)
                    nc.vector.scalar_tensor_tensor(
                        scr[:], Xt[:, c, sl], 1.0, ebs_[rh][:],
                        Alu.mult, Alu.mult,
                        accum_out=vh[:, 2 * c + rh:2 * c + rh + 1])

            # ---- pairwise AllReduce of (v, l2) ----
            vsum = sb.tile([128, NK], f32, tag="vsum", bufs=2)
            vh3 = vh[:].rearrange("p (c t) -> p c t", t=2)
            nc.vector.tensor_tensor(vsum[:], vh3[:, :, 0], vh3[:, :, 1], Alu.add)
            colsb = sb.tile([128, 16], f32, tag="colsb", bufs=2)
            nc.vector.memset(colsb[:], 0.0)
            nc.vector.tensor_copy(colsb[:, 0:NK], vsum[:])
            nc.vector.tensor_copy(colsb[0:1, NK:NK + 1], l2s[:])
            cin = dram.tile([128, 16], f32, tag="cin")
            cout = dram.tile([128, 16], f32, tag="cout")
            nc.sync.dma_start(cin[:], colsb[:])
            nc.gpsimd.collective_compute(
                "AllReduce", Alu.add,
                replica_groups=[[0, 1], [2, 3], [4, 5], [6, 7]],
                ins=[cin[:].opt()], outs=[cout[:].opt()])
            colg = sb.tile([128, 16], f32, tag="colg", bufs=2)
            nc.sync.dma_start(colg[:], cout[:])

            # rank-64 path part 1: Rw2 accumulation (no collective dependency)
            ps_r = []
            for hf in range(RH):
                slh = slice(hf * 512, (hf + 1) * 512)
                ps_ri = ps_bc.tile([Q_LEN, 512], f32, tag="bc")
                for c in range(NK):
                    nc.tensor.matmul(ps_ri[:], mT[:, c], w2ch[c][:, slh],
                                     start=(c == 0), stop=False)
                ps_r.append(ps_ri)

            # ---- main W1/W3 blocks ----
            for j in range(1, NK):
                load_w13(j)
                w1j, w3j = w13[j]
                for rh in range(RH):
                    sl = slice(rh * 512, (rh + 1) * 512)
                    ps_ab = ps_jg.tile([128, 512], f32, tag="jg")
                    for c in range(NK):
                        nc.tensor.matmul(ps_ab[:], w1j[:, c], Xt[:, c, sl],
                                         start=(c == 0), stop=False)
                    for c in range(NK):
                        nc.tensor.matmul(ps_ab[:], w3j[:, c], XO[:, c, sl],
                                         start=False, stop=(c == NK - 1))
                    nc.scalar.copy(oT[:, j, sl], ps_ab[:])

            # ---- collective-dependent tail ----
            linv = sb.tile([1, 1], bf16, tag="linv", bufs=2)
            with nc.allow_low_precision(reason="weight-two scale in bf16"):
                nc.vector.reciprocal(linv[:], colg[0:1, NK:NK + 1])
            ps_tb = ps_tp.tile([128, 1], f32, tag="tp")
            nc.tensor.matmul(ps_tb[:], ones_r[:], linv[:], start=True, stop=True)
            tvec = sb.tile([128, NK], f32, tag="tvec", bufs=2)
            nc.vector.tensor_scalar(tvec[:], colg[:, 0:NK], ps_tb[:, 0:1],
                                    None, Alu.mult)
            mTs = sb.tile([128, NK, Q_LEN], bf16, tag="mTs", bufs=2)
            for c in range(NK):
                nc.vector.tensor_scalar(mTs[:, c], mT[:, c], tvec[:, c:c + 1],
                                        None, Alu.mult)

            # rank-64 path part 2: += (M*t)@W4, then Rsb = psum
            Rsb = sb.tile([Q_LEN, D], bf16, tag="Rsb", bufs=2)
            for hf in range(RH):
                slh = slice(hf * 512, (hf + 1) * 512)
                for c in range(NK):
                    nc.tensor.matmul(ps_r[hf][:], mTs[:, c], w4ch[c][:, slh],
                                     start=False, stop=(c == NK - 1))
                nc.scalar.copy(Rsb[:, slh], ps_r[hf][:])

            # rank-64 correction (rh-major so the next stage starts sooner)
            for rh in range(RH):
                sl = slice(rh * 512, (rh + 1) * 512)
                for j in range(NK):
                    ps_c = ps_bc.tile([128, 512], f32, tag="bc")
                    nc.tensor.matmul(ps_c[:], Rsb[:, j * 128:(j + 1) * 128],
                                     P[:, sl], start=True, stop=True)
                    if j % 2 == 0:
                        nc.vector.tensor_tensor(oT[:, j, sl], oT[:, j, sl],
                                                ps_c[:], Alu.add)
                    else:
                        # Pool can't read PSUM: stage through SBUF via Act
                        cs = sb.tile([128, 512], bf16, tag="corrs", bufs=2)
                        nc.scalar.copy(cs[:], ps_c[:])
                        nc.gpsimd.tensor_tensor(oT[:, j, sl], oT[:, j, sl],
                                                cs[:], Alu.add)
            return oT

        o1T = run_stage(0, xt0)
        o2T = run_stage(1, o1T)

        # ---------- final linear (transposed): outT = w_mapT.T @ o2T + b ----
        for rh in range(RH):
            sl = slice(rh * 512, (rh + 1) * 512)
            for j2 in range(NJ2):
                ps_f = ps_jg.tile([128, 512], f32, tag="jg")
                for c in range(NK):
                    nc.tensor.matmul(ps_f[:], wmjs[j2][:, c], o2T[:, c, sl],
                                     start=(c == 0), stop=(c == NK - 1))
                outsb = sb.tile([128, 512], bf16, tag="outsb", bufs=4)
                if j2 % 2 == 0:
                    nc.scalar.activation(outsb[:], ps_f[:], AF.Identity,
                                         bias=bcols[:, j2:j2 + 1], scale=1.0)
                else:
                    nc.vector.tensor_scalar(outsb[:], ps_f[:],
                                            bcols[:, j2:j2 + 1], None, Alu.add)
                q = nc.sync if j2 % 2 == 0 else nc.scalar
                q.dma_start(out_ap[j2 * 128:(j2 + 1) * 128, sl], outsb[:])

    nc.compile()
    return nc


def _get_nc():
    global _CACHED_NC
    if _CACHED_NC is None:
        _CACHED_NC = _build_nc()
    return _CACHED_NC


def _shard_inputs(inputs):
    """Build the 8 per-core input maps (layout + dtype cast only)."""
    x = np.asarray(inputs["ctx_features"], dtype=np.float32)
    q1 = np.asarray(inputs["sub_q1_features"], dtype=np.float32)
    q2 = np.asarray(inputs["sub_q2_features"], dtype=np.float32)
    k1 = np.ascontiguousarray(np.asarray(inputs["sub_q1_attn_mask"], dtype=np.int32))
    k2 = np.ascontiguousarray(np.asarray(inputs["sub_q2_attn_mask"], dtype=np.int32))

    def wblocks(w_out):
        # w_out [D, 4D] -> wb = w_out.T [4D, D]; W_k = wb[kD:(k+1)D]
        wb = np.ascontiguousarray(w_out.T.astype(BF))
        W1, W2, W3, W4 = (wb[k * D:(k + 1) * D] for k in range(4))

        def jmaj(W):  # j-major tiling for the lhsT stream
            return np.ascontiguousarray(
                W.reshape(NK, 128, NK, 128).transpose(2, 1, 0, 3).reshape(D, D))
        return jmaj(W1), np.ascontiguousarray(W2), jmaj(W3), np.ascontiguousarray(W4)

    w1t1, w2c1, w3t1, w4c1 = wblocks(inputs["w_out1"])
    w1t2, w2c2, w3t2, w4c2 = wblocks(inputs["w_out2"])

    wmT = inputs["w_map"].T.astype(BF)  # [D, 2D]
    wmt = np.ascontiguousarray(
        wmT.reshape(NK, 128, NJ2, 128).transpose(2, 1, 0, 3).reshape(D2, D))
    bmap = np.ascontiguousarray(
        np.asarray(inputs["b_map"], dtype=np.float32).reshape(NJ2, 128).T)

    def ptile_vec(*cols):  # [D] vectors -> [128, NK*k] p-major
        v = np.stack([np.asarray(c, dtype=np.float32) for c in cols], axis=-1)
        k = v.shape[-1]
        return np.ascontiguousarray(
            v.reshape(NK, 128, k).transpose(1, 0, 2).reshape(128, NK * k))

    def pmaj(vecd, dt=BF):  # [D] -> [128, NK] p-major
        return np.ascontiguousarray(
            np.asarray(vecd).reshape(NK, 128).T.astype(dt))

    stage_common = {
        "vec1": ptile_vec(inputs["w_in1"], inputs["w_mem1"], inputs["scale1"]),
        "vec2": ptile_vec(inputs["w_in2"], inputs["w_mem2"], inputs["scale2"]),
        "wmb1": pmaj(inputs["w_mem1"]),
        "wmb2": pmaj(inputs["w_mem2"]),
        "w1t1": w1t1, "w3t1": w3t1, "w2c1": w2c1, "w4c1": w4c1,
        "w1t2": w1t2, "w3t2": w3t2, "w2c2": w2c2, "w4c2": w4c2,
        "wmt": wmt, "bmap": bmap,
    }

    in_maps = []
    for core in range(N_CORES):
        b, h = divmod(core, 2)
        xT = x[b, h * R:(h + 1) * R, :].T.astype(BF)  # [D, R]
        xt_tile = np.ascontiguousarray(
            xT.reshape(NK, 128, R).transpose(1, 0, 2).reshape(128, NK * R))
        m = {}
        for s, q, kk in ((1, q1, k1), (2, q2, k2)):
            mT = q[b].T.astype(BF)  # [D, Q]
            m[f"m{s}t"] = np.ascontiguousarray(
                mT.reshape(NK, 128, Q_LEN).transpose(1, 0, 2).reshape(128, NK * Q_LEN))
            m[f"m{s}n"] = np.ascontiguousarray(q[b].astype(BF))
            m[f"mask{s}"] = np.ascontiguousarray(kk[b].reshape(Q_LEN, 1))
        in_maps.append({"xt": xt_tile, **m, **stage_common})
    return in_maps


def _gather_outputs(results):
    out = np.empty((B, C_LEN, D2), dtype=np.float32)
    for core in range(N_CORES):
        b, h = divmod(core, 2)
        out[b, h * R:(h + 1) * R, :] = results[core]["out"].astype(np.float32).T
    return out


def kernel(**inputs):
    nc = _get_nc()
    in_maps = _shard_inputs(inputs)
    last_err = None
    for _attempt in range(3):
        try:
            res = run_bass_kernel_spmd(nc, in_maps, core_ids=list(range(N_CORES)))
            return _gather_outputs(res.results)
        except Exception as e:  # transient device errors: retry
            last_err = e
    raise last_err


# revision 14
# speedup vs baseline: 1.4393x; 1.2238x over previous
"""Trainium2 Bass kernel for nn_NewModel_42356967473589 (dense_transformer).

Model: two BiAttention blocks + final linear mapping.
  o = BiAttn(ctx, q1) ; o = BiAttn(o, q2) ; out = o @ w_map.T + b_map

Sharding: 8 cores = (batch b in 0..3) x (context half h in 0..1).
Each core owns 1024 context rows of one batch. All compute is row-local
except the softmax-over-context (weight_two); its (sum-exp, weighted-sum)
stats are combined across the pair of cores sharing a batch via a tiny
pairwise AllReduce, overlapped with the large matmuls.

Math restructure (per stage, X = stage input [C,D], M = memory [Q,D]):
  out = X@W1 + o1@W2 + (X*o1)@W3 + (t*o1)@W4      (W_k = w_out[:, kD:(k+1)D].T)
  o1 = P@M (rank Q=64), t broadcast over rows =>
  o1@W2 + (t*o1)@W4 = P @ (M@W2 + (M*t)@W4)        (rank-64 path)

All big operands are bf16 (matmul rate on TRN2 is the same as f32r at
>=256 moving rows, but DMA/SBUF/DVE traffic halves); accumulation stays in
fp32 PSUM and softmax statistics are computed in fp32. The column-softmax
max/sum stats are obtained by PE-transposing the exp'd score tile and
reducing along the free axis on DVE (the gpsimd partition_all_reduce the
earlier version used was ~16.5us per call and serialized the stage).
"""

import numpy as np
import ml_dtypes

import concourse.bacc as bacc
import concourse.tile as tile
from concourse import mybir
from concourse.bass_utils import run_bass_kernel_spmd
from concourse.masks import make_identity
from contextlib import ExitStack
import bass_rust

f32 = mybir.dt.float32
bf16 = mybir.dt.bfloat16
i32 = mybir.dt.int32
Alu = mybir.AluOpType
AF = bass_rust.ActivationFunctionType
AX = bass_rust.AxisListType

B, C_LEN, Q_LEN, D = 4, 2048, 64, 1024
N_CORES = 8
R = C_LEN // 2          # rows per core
NK = D // 128           # contraction chunks
RH = R // 512           # row halves (moving-dim tiles)
NJ2 = 16                # output blocks of the final linear
D2 = 2 * D
NEGBIG = 10000.0
BF = ml_dtypes.bfloat16

_CACHED_NC = None


def _build_nc():
    nc = bacc.Bacc("TRN2", target_bir_lowering=False, debug=False,
                   num_devices=N_CORES)

    # ---- per-core DRAM I/O (host pre-tiled layouts, see _shard_inputs) ----
    xt_ap = nc.dram_tensor("xt", [128, NK * R], bf16, kind="ExternalInput").ap()
    m_t = [nc.dram_tensor(f"m{s}t", [128, NK * Q_LEN], bf16, kind="ExternalInput").ap() for s in (1, 2)]
    m_n = [nc.dram_tensor(f"m{s}n", [Q_LEN, D], bf16, kind="ExternalInput").ap() for s in (1, 2)]
    vec = [nc.dram_tensor(f"vec{s}", [128, NK * 3], f32, kind="ExternalInput").ap() for s in (1, 2)]
    wmb = [nc.dram_tensor(f"wmb{s}", [128, NK], bf16, kind="ExternalInput").ap() for s in (1, 2)]
    msk = [nc.dram_tensor(f"mask{s}", [Q_LEN, 1], i32, kind="ExternalInput").ap() for s in (1, 2)]
    w1t = [nc.dram_tensor(f"w1t{s}", [D, D], bf16, kind="ExternalInput").ap() for s in (1, 2)]
    w3t = [nc.dram_tensor(f"w3t{s}", [D, D], bf16, kind="ExternalInput").ap() for s in (1, 2)]
    w2c = [nc.dram_tensor(f"w2c{s}", [D, D], bf16, kind="ExternalInput").ap() for s in (1, 2)]
    w4c = [nc.dram_tensor(f"w4c{s}", [D, D], bf16, kind="ExternalInput").ap() for s in (1, 2)]
    wmt_ap = nc.dram_tensor("wmt", [D2, D], bf16, kind="ExternalInput").ap()
    bmap_ap = nc.dram_tensor("bmap", [128, NJ2], f32, kind="ExternalInput").ap()
    out_ap = nc.dram_tensor("out", [D2, R], bf16, kind="ExternalOutput").ap()

    with tile.TileContext(nc) as tc, ExitStack() as ctx:
        sb = ctx.enter_context(tc.tile_pool(name="sb", bufs=1))
        sb_xt = ctx.enter_context(tc.tile_pool(name="sb_xt", bufs=2))
        sb_xo = ctx.enter_context(tc.tile_pool(name="sb_xo", bufs=1))
        sb_w13 = ctx.enter_context(tc.tile_pool(name="sb_w13", bufs=16))
        sb_wmj = ctx.enter_context(tc.tile_pool(name="sb_wmj", bufs=16))
        sb_w24 = ctx.enter_context(tc.tile_pool(name="sb_w24", bufs=8))
        ps_sc = ctx.enter_context(tc.tile_pool(name="ps_sc", bufs=2, space="PSUM"))
        ps_tp = ctx.enter_context(tc.tile_pool(name="ps_tp", bufs=2, space="PSUM"))
        ps_bc = ctx.enter_context(tc.tile_pool(name="ps_bc", bufs=2, space="PSUM"))
        ps_jg = ctx.enter_context(tc.tile_pool(name="ps_jg", bufs=2, space="PSUM"))
        dram = ctx.enter_context(tc.tile_pool(name="dram", bufs=2, space="DRAM"))

        # ---- constants ----
        ident = sb.tile([128, 128], bf16, tag="ident")
        make_identity(nc, ident[:])
        ones_r = sb.tile([1, 128], bf16, tag="ones_r")
        nc.vector.memset(ones_r[:], 1.0)
        ones_c = sb.tile([128, 1], bf16, tag="ones_c")
        nc.vector.memset(ones_c[:], 1.0)

        # ---- stage input 1: X^T (software-DGE queue) ----
        xt0 = sb_xt.tile([128, NK, R], bf16, tag="xt")
        for c in range(NK):
            nc.gpsimd.dma_start(xt0[:, c], xt_ap[:, c * R:(c + 1) * R])

        # ---- per-stage constants + memory_dot / mbias, both stages up front
        st = [dict() for _ in range(2)]

        def prep_stage(s):
            d = st[s]
            vecs = sb.tile([128, NK, 3], f32, tag=f"vecs{s}")
            nc.sync.dma_start(vecs[:], vec[s][:].rearrange("p (c k) -> p c k", c=NK))
            mT = sb.tile([128, NK, Q_LEN], bf16, tag=f"mT{s}")
            nc.sync.dma_start(mT[:], m_t[s][:].rearrange("p (c q) -> p c q", c=NK))
            wmbs = sb.tile([128, NK], bf16, tag=f"wmb{s}")
            nc.sync.dma_start(wmbs[:], wmb[s][:])
            mN = sb.tile([Q_LEN, D], bf16, tag=f"mN{s}")
            nc.sync.dma_start(mN[:], m_n[s][:])
            mask_i = sb.tile([Q_LEN, 1], i32, tag=f"mask{s}")
            nc.sync.dma_start(mask_i[:], msk[s][:])

            # mst = [M^T * scale | w_in]  (lhsT for the score matmul)
            mst = sb.tile([128, NK, Q_LEN + 1], bf16, tag=f"mst{s}")
            for c in range(NK):
                nc.vector.tensor_scalar(mst[:, c, 0:Q_LEN], mT[:, c],
                                        vecs[:, c, 2:3], None, Alu.mult)
            nc.vector.tensor_copy(mst[:, :, Q_LEN:Q_LEN + 1], vecs[:, :, 0:1])

            # memory_dot = M @ w_mem  -> psum [Q,1]
            ps_md = ps_tp.tile([Q_LEN, 1], f32, tag="tp")
            for c in range(NK):
                nc.tensor.matmul(ps_md[:], mT[:, c], wmbs[:, c:c + 1],
                                 start=(c == 0), stop=(c == NK - 1))
            maskf = sb.tile([Q_LEN, 1], f32, tag=f"maskf{s}")
            nc.vector.tensor_copy(maskf[:], mask_i[:])
            mbias = sb.tile([Q_LEN, 1], f32, tag=f"mbias{s}")
            nc.vector.tensor_scalar(mbias[:], maskf[:], NEGBIG, -NEGBIG,
                                    Alu.mult, Alu.add)
            nc.vector.tensor_tensor(mbias[:], mbias[:], ps_md[:], Alu.add)
            d.update(mT=mT, mN=mN, mst=mst, mbias=mbias)

        def load_w24(s):
            # software-DGE queue: keeps the Act hwdge queue free for compute
            w2ch, w4ch = [], []
            for c in range(NK):
                w2h = sb_w24.tile([128, 1024], bf16, tag="w2h")
                nc.gpsimd.dma_start(w2h[:], w2c[s][c * 128:(c + 1) * 128, :])
                w2ch.append(w2h)
            for c in range(NK):
                w4h = sb_w24.tile([128, 1024], bf16, tag="w4h")
                nc.gpsimd.dma_start(w4h[:], w4c[s][c * 128:(c + 1) * 128, :])
                w4ch.append(w4h)
            st[s].update(w2ch=w2ch, w4ch=w4ch)

        prep_stage(0)
        prep_stage(1)
        load_w24(0)

        # final-linear weights: all 16 blocks resident (software-DGE queue)
        wmjs = []
        for j2 in range(NJ2):
            wmj = sb_wmj.tile([128, NK, 128], bf16, tag="wmj")
            nc.gpsimd.dma_start(wmj[:], wmt_ap[j2 * 128:(j2 + 1) * 128, :]
                                .rearrange("p (c m) -> p c m", c=NK))
            wmjs.append(wmj)
        bcols = sb.tile([128, NJ2], f32, tag="bcols")
        nc.scalar.dma_start(bcols[:], bmap_ap[:])

        def run_stage(s, Xt):
            """One BiAttention stage; returns o^T tile [128, NK, R] bf16."""
            d = st[s]
            mT, mN, mst, mbias = d["mT"], d["mN"], d["mst"], d["mbias"]
            w2ch, w4ch = d["w2ch"], d["w4ch"]

            w13 = {}

            def load_w13(j):
                w1j = sb_w13.tile([128, NK, 128], bf16, tag="w13")
                nc.sync.dma_start(w1j[:], w1t[s][j * 128:(j + 1) * 128, :]
                                  .rearrange("p (c m) -> p c m", c=NK))
                w3j = sb_w13.tile([128, NK, 128], bf16, tag="w13")
                nc.sync.dma_start(w3j[:], w3t[s][j * 128:(j + 1) * 128, :]
                                  .rearrange("p (c m) -> p c m", c=NK))
                w13[j] = (w1j, w3j)

            for j in range(NK):
                load_w13(j)

            P = sb.tile([Q_LEN, R], bf16, tag="P", bufs=2)
            st8 = sb.tile([128, RH, 8], bf16, tag="st8", bufs=2)
            vh = sb.tile([128, 2 * NK], f32, tag="vh", bufs=2)
            E_ = [None] * RH
            TP_ = [None] * RH
            rs_ = [None] * RH
            ebs_ = [None] * RH

            def scores(rh):
                sl = slice(rh * 512, (rh + 1) * 512)
                ps = ps_sc.tile([Q_LEN + 1, 512], f32, tag="sc")
                for c in range(NK):
                    nc.tensor.matmul(ps[:], mst[:, c], Xt[:, c, sl],
                                     start=(c == 0), stop=(c == NK - 1))
                E = sb.tile([Q_LEN + 1, 512], bf16, tag="E", bufs=2)
                nc.scalar.activation(E[0:Q_LEN], ps[0:Q_LEN], AF.Exp,
                                     bias=mbias[:], scale=1.0)
                nc.scalar.activation(E[Q_LEN:Q_LEN + 1], ps[Q_LEN:Q_LEN + 1],
                                     AF.Exp)
                E_[rh] = E

            def transposes(rh):
                # E chunks [65,128] -> [128,65] so q lands on the free axis
                TP = ps_tp.tile([128, 4, 80], bf16, tag="tp")
                for k in range(4):
                    nc.tensor.transpose(TP[:, k, 0:Q_LEN + 1],
                                        E_[rh][:, k * 128:(k + 1) * 128],
                                        ident[0:Q_LEN + 1, 0:Q_LEN + 1])
                TP_[rh] = TP

            def stats(rh):
                TP = TP_[rh]
                mx = sb.tile([128, 4], f32, tag="mx", bufs=2)
                nc.vector.reduce_max(mx[:], TP[:, :, 0:Q_LEN], AX.X)
                l1 = sb.tile([128, 4], f32, tag="l1", bufs=2)
                nc.vector.reduce_sum(l1[:], TP[:, :, 0:Q_LEN], AX.X)
                # e2 = (max_q E) * exp(input_dot); l1r = 1/l1
                nc.vector.tensor_tensor(st8[:, rh, 0:4], mx[:],
                                        TP[:, :, Q_LEN], Alu.mult)
                with nc.allow_low_precision(reason="softmax scales in bf16"):
                    nc.vector.reciprocal(st8[:, rh, 4:8], l1[:])

            def stats_rows(rh):
                # column stats [128,{e2|l1r}x4] -> row form [1, 1024] on p0
                tp8 = ps_tp.tile([1, 1024], bf16, tag="tp")
                for k in range(8):
                    nc.tensor.transpose(tp8[0:1, k * 128:(k + 1) * 128],
                                        st8[:, rh, k:k + 1], ident[:])
                rs = sb.tile([1, 1024], bf16, tag="rowst", bufs=2)
                nc.scalar.copy(rs[0:1, 0:512], tp8[0:1, 0:512])
                nc.vector.tensor_copy(rs[0:1, 512:1024], tp8[0:1, 512:1024])
                rs_[rh] = rs

            def bcasts(rh):
                rs = rs_[rh]
                eb = ps_bc.tile([128, 512], f32, tag="bc")
                for k in range(4):
                    nc.tensor.matmul(eb[:, k * 128:(k + 1) * 128], ones_r[:],
                                     rs[0:1, k * 128:(k + 1) * 128],
                                     start=True, stop=True)
                ebs = sb.tile([128, 512], bf16, tag="ebs", bufs=2)
                nc.scalar.copy(ebs[:], eb[:])
                ebs_[rh] = ebs
                rb = ps_bc.tile([Q_LEN, 512], f32, tag="bc")
                for k in range(4):
                    nc.tensor.matmul(rb[:, k * 128:(k + 1) * 128],
                                     ones_r[:, 0:Q_LEN],
                                     rs[0:1, (4 + k) * 128:(5 + k) * 128],
                                     start=True, stop=True)
                sl = slice(rh * 512, (rh + 1) * 512)
                nc.vector.tensor_tensor(P[:, sl], E_[rh][0:Q_LEN], rb[:],
                                        Alu.mult)

            # ---- PE schedule: scores/stats with pre-opened j=0 groups ----
            pre = []
            scores(0)
            for rh in range(RH):
                sl = slice(rh * 512, (rh + 1) * 512)
                pj = ps_jg.tile([128, 512], f32, tag="jg")
                for c in range(NK):
                    nc.tensor.matmul(pj[:], w13[0][0][:, c], Xt[:, c, sl],
                                     start=(c == 0), stop=False)
                pre.append(pj)
                if rh == 0:
                    transposes(0)
                    stats(0)
            scores(1)
            stats_rows(0)
            bcasts(0)
            transposes(1)
            stats(1)
            stats_rows(1)
            bcasts(1)

            # l2 = sum over all rows of e2  (both rh at once)
            ps_l2 = ps_tp.tile([1, RH * 4], f32, tag="tp")
            nc.tensor.matmul(ps_l2[:], ones_c[:], st8[:, :, 0:4],
                             start=True, stop=True)
            l2s = sb.tile([1, 1], f32, tag="l2s", bufs=2)
            nc.vector.reduce_sum(l2s[:], ps_l2[:], AX.X)

            # ---- o1^T = mN.T @ P and XO = Xt * o1 ----
            XO = sb_xo.tile([128, NK, R], bf16, tag="xo")
            for c in range(NK):
                for rh in range(RH):
                    sl = slice(rh * 512, (rh + 1) * 512)
                    ps_o1 = ps_bc.tile([128, 512], f32, tag="bc")
                    nc.tensor.matmul(ps_o1[:], mN[:, c * 128:(c + 1) * 128],
                                     P[:, sl], start=True, stop=True)
                    if c % 2 == 0:
                        nc.vector.tensor_tensor(XO[:, c, sl], Xt[:, c, sl],
                                                ps_o1[:], Alu.mult)
                    else:
                        # spread work: stage PSUM->SBUF via Act, multiply on Pool
                        o1s = sb.tile([128, 512], bf16, tag="o1s", bufs=2)
                        nc.scalar.copy(o1s[:], ps_o1[:])
                        nc.gpsimd.tensor_tensor(XO[:, c, sl], Xt[:, c, sl],
                                                o1s[:], Alu.mult)

            oT = sb_xt.tile([128, NK, R], bf16, tag="xt")
            # close the pre-opened j=0 groups with their W3 halves
            for rh, pj in enumerate(pre):
                sl = slice(rh * 512, (rh + 1) * 512)
                for c in range(NK):
                    nc.tensor.matmul(pj[:], w13[0][1][:, c], XO[:, c, sl],
                                     start=False, stop=(c == NK - 1))
                nc.scalar.copy(oT[:, 0, sl], pj[:])

            # v = sum over rows of e2*X  (accumulated on DVE+Pool from ebs)
            for rh in range(RH):
                sl = slice(rh * 512, (rh + 1) * 512)
                for c in range(NK):
                    scr = sb.tile([128, 512], bf16, tag="scrV", bufs=2)
                    nc.vector.scalar_tensor_tensor(
                        scr[:], Xt[:, c, sl], 1.0, ebs_[rh][:],
                        Alu.mult, Alu.mult,
                        accum_out=vh[:, 2 * c + rh:2 * c + rh + 1])

            # ---- pairwise AllReduce of (v, l2) ----
            vsum = sb.tile([128, NK], f32, tag="vsum", bufs=2)
            vh3 = vh[:].rearrange("p (c t) -> p c t", t=2)
            nc.vector.tensor_tensor(vsum[:], vh3[:, :, 0], vh3[:, :, 1], Alu.add)
            colsb = sb.tile([128, 16], f32, tag="colsb", bufs=2)
            nc.vector.memset(colsb[:], 0.0)
            nc.vector.tensor_copy(colsb[:, 0:NK], vsum[:])
            nc.vector.tensor_copy(colsb[0:1, NK:NK + 1], l2s[:])
            cin = dram.tile([128, 16], f32, tag="cin")
            cout = dram.tile([128, 16], f32, tag="cout")
            nc.sync.dma_start(cin[:], colsb[:])
            nc.gpsimd.collective_compute(
                "AllReduce", Alu.add,
                replica_groups=[[0, 1], [2, 3], [4, 5], [6, 7]],
                ins=[cin[:].opt()], outs=[cout[:].opt()])
            colg = sb.tile([128, 16], f32, tag="colg", bufs=2)
            nc.sync.dma_start(colg[:], cout[:])

            # rank-64 path part 1: Rw2 accumulation (no collective dependency)
            ps_r = []
            for hf in range(RH):
                slh = slice(hf * 512, (hf + 1) * 512)
                ps_ri = ps_bc.tile([Q_LEN, 512], f32, tag="bc")
                for c in range(NK):
                    nc.tensor.matmul(ps_ri[:], mT[:, c], w2ch[c][:, slh],
                                     start=(c == 0), stop=False)
                ps_r.append(ps_ri)

            # ---- main W1/W3 blocks ----
            for j in range(1, NK):
                w1j, w3j = w13[j]
                for rh in range(RH):
                    sl = slice(rh * 512, (rh + 1) * 512)
                    ps_ab = ps_jg.tile([128, 512], f32, tag="jg")
                    for c in range(NK):
                        nc.tensor.matmul(ps_ab[:], w1j[:, c], Xt[:, c, sl],
                                         start=(c == 0), stop=False)
                    for c in range(NK):
                        nc.tensor.matmul(ps_ab[:], w3j[:, c], XO[:, c, sl],
                                         start=False, stop=(c == NK - 1))
                    nc.scalar.copy(oT[:, j, sl], ps_ab[:])

            # ---- collective-dependent tail ----
            linv = sb.tile([1, 1], bf16, tag="linv", bufs=2)
            with nc.allow_low_precision(reason="weight-two scale in bf16"):
                nc.vector.reciprocal(linv[:], colg[0:1, NK:NK + 1])
            ps_tb = ps_tp.tile([128, 1], f32, tag="tp")
            nc.tensor.matmul(ps_tb[:], ones_r[:], linv[:], start=True, stop=True)
            tvec = sb.tile([128, NK], f32, tag="tvec", bufs=2)
            nc.vector.tensor_scalar(tvec[:], colg[:, 0:NK], ps_tb[:, 0:1],
                                    None, Alu.mult)
            mTs = sb.tile([128, NK, Q_LEN], bf16, tag="mTs", bufs=2)
            for c in range(NK):
                nc.vector.tensor_scalar(mTs[:, c], mT[:, c], tvec[:, c:c + 1],
                                        None, Alu.mult)

            # rank-64 path part 2: += (M*t)@W4, then Rsb = psum
            Rsb = sb.tile([Q_LEN, D], bf16, tag="Rsb", bufs=2)
            for hf in range(RH):
                slh = slice(hf * 512, (hf + 1) * 512)
                for c in range(NK):
                    nc.tensor.matmul(ps_r[hf][:], mTs[:, c], w4ch[c][:, slh],
                                     start=False, stop=(c == NK - 1))
                nc.scalar.copy(Rsb[:, slh], ps_r[hf][:])

            # rank-64 correction (rh-major so the next stage starts sooner)
            for rh in range(RH):
                sl = slice(rh * 512, (rh + 1) * 512)
                for j in range(NK):
                    ps_c = ps_bc.tile([128, 512], f32, tag="bc")
                    nc.tensor.matmul(ps_c[:], Rsb[:, j * 128:(j + 1) * 128],
                                     P[:, sl], start=True, stop=True)
                    if j % 2 == 0:
                        nc.vector.tensor_tensor(oT[:, j, sl], oT[:, j, sl],
                                                ps_c[:], Alu.add)
                    else:
                        # Pool can't read PSUM: stage through SBUF via Act
                        cs = sb.tile([128, 512], bf16, tag="corrs", bufs=2)
                        nc.scalar.copy(cs[:], ps_c[:])
                        nc.gpsimd.tensor_tensor(oT[:, j, sl], oT[:, j, sl],
                                                cs[:], Alu.add)
            return oT

        o1T = run_stage(0, xt0)
        load_w24(1)
        o2T = run_stage(1, o1T)

        # ---------- final linear (transposed): outT = w_mapT.T @ o2T + b ----
        for rh in range(RH):
            sl = slice(rh * 512, (rh + 1) * 512)
            for j2 in range(NJ2):
                ps_f = ps_jg.tile([128, 512], f32, tag="jg")
                for c in range(NK):
                    nc.tensor.matmul(ps_f[:], wmjs[j2][:, c], o2T[:, c, sl],
                                     start=(c == 0), stop=(c == NK - 1))
                outsb = sb.tile([128, 512], bf16, tag="outsb", bufs=4)
                if j2 % 2 == 0:
                    nc.scalar.activation(outsb[:], ps_f[:], AF.Identity,
                                         bias=bcols[:, j2:j2 + 1], scale=1.0)
                else:
                    nc.vector.tensor_scalar(outsb[:], ps_f[:],
                                            bcols[:, j2:j2 + 1], None, Alu.add)
                q = nc.sync if j2 % 2 == 0 else nc.scalar
                q.dma_start(out_ap[j2 * 128:(j2 + 1) * 128, sl], outsb[:])

    nc.compile()
    return nc


def _get_nc():
    global _CACHED_NC
    if _CACHED_NC is None:
        _CACHED_NC = _build_nc()
    return _CACHED_NC


def _shard_inputs(inputs):
    """Build the 8 per-core input maps (layout + dtype cast only)."""
    x = np.asarray(inputs["ctx_features"], dtype=np.float32)
    q1 = np.asarray(inputs["sub_q1_features"], dtype=np.float32)
    q2 = np.asarray(inputs["sub_q2_features"], dtype=np.float32)
    k1 = np.ascontiguousarray(np.asarray(inputs["sub_q1_attn_mask"], dtype=np.int32))
    k2 = np.ascontiguousarray(np.asarray(inputs["sub_q2_attn_mask"], dtype=np.int32))

    def wblocks(w_out):
        # w_out [D, 4D] -> wb = w_out.T [4D, D]; W_k = wb[kD:(k+1)D]
        wb = np.ascontiguousarray(w_out.T.astype(BF))
        W1, W2, W3, W4 = (wb[k * D:(k + 1) * D] for k in range(4))

        def jmaj(W):  # j-major tiling for the lhsT stream
            return np.ascontiguousarray(
                W.reshape(NK, 128, NK, 128).transpose(2, 1, 0, 3).reshape(D, D))
        return jmaj(W1), np.ascontiguousarray(W2), jmaj(W3), np.ascontiguousarray(W4)

    w1t1, w2c1, w3t1, w4c1 = wblocks(inputs["w_out1"])
    w1t2, w2c2, w3t2, w4c2 = wblocks(inputs["w_out2"])

    wmT = inputs["w_map"].T.astype(BF)  # [D, 2D]
    wmt = np.ascontiguousarray(
        wmT.reshape(NK, 128, NJ2, 128).transpose(2, 1, 0, 3).reshape(D2, D))
    bmap = np.ascontiguousarray(
        np.asarray(inputs["b_map"], dtype=np.float32).reshape(NJ2, 128).T)

    def ptile_vec(*cols):  # [D] vectors -> [128, NK*k] p-major
        v = np.stack([np.asarray(c, dtype=np.float32) for c in cols], axis=-1)
        k = v.shape[-1]
        return np.ascontiguousarray(
            v.reshape(NK, 128, k).transpose(1, 0, 2).reshape(128, NK * k))

    def pmaj(vecd, dt=BF):  # [D] -> [128, NK] p-major
        return np.ascontiguousarray(
            np.asarray(vecd).reshape(NK, 128).T.astype(dt))

    stage_common = {
        "vec1": ptile_vec(inputs["w_in1"], inputs["w_mem1"], inputs["scale1"]),
        "vec2": ptile_vec(inputs["w_in2"], inputs["w_mem2"], inputs["scale2"]),
        "wmb1": pmaj(inputs["w_mem1"]),
        "wmb2": pmaj(inputs["w_mem2"]),
        "w1t1": w1t1, "w3t1": w3t1, "w2c1": w2c1, "w4c1": w4c1,
        "w1t2": w1t2, "w3t2": w3t2, "w2c2": w2c2, "w4c2": w4c2,
        "wmt": wmt, "bmap": bmap,
    }

    in_maps = []
    for core in range(N_CORES):
        b, h = divmod(core, 2)
        xT = x[b, h * R:(h + 1) * R, :].T.astype(BF)  # [D, R]
        xt_tile = np.ascontiguousarray(
            xT.reshape(NK, 128, R).transpose(1, 0, 2).reshape(128, NK * R))
        m = {}
        for s, q, kk in ((1, q1, k1), (2, q2, k2)):
            mT = q[b].T.astype(BF)  # [D, Q]
            m[f"m{s}t"] = np.ascontiguousarray(
                mT.reshape(NK, 128, Q_LEN).transpose(1, 0, 2).reshape(128, NK * Q_LEN))
            m[f"m{s}n"] = np.ascontiguousarray(q[b].astype(BF))
            m[f"mask{s}"] = np.ascontiguousarray(kk[b].reshape(Q_LEN, 1))
        in_maps.append({"xt": xt_tile, **m, **stage_common})
    return in_maps


def _gather_outputs(results):
    out = np.empty((B, C_LEN, D2), dtype=np.float32)
    for core in range(N_CORES):
        b, h = divmod(core, 2)
        out[b, h * R:(h + 1) * R, :] = results[core]["out"].astype(np.float32).T
    return out


def kernel(**inputs):
    nc = _get_nc()
    in_maps = _shard_inputs(inputs)
    last_err = None
    for _attempt in range(3):
        try:
            res = run_bass_kernel_spmd(nc, in_maps, core_ids=list(range(N_CORES)))
            return _gather_outputs(res.results)
        except Exception as e:  # transient device errors: retry
            last_err = e
    raise last_err


# revision 15
# speedup vs baseline: 1.5494x; 1.0765x over previous
"""Trainium2 Bass kernel for nn_NewModel_42356967473589 (dense_transformer).

Model: two BiAttention blocks + final linear mapping.
  o = BiAttn(ctx, q1) ; o = BiAttn(o, q2) ; out = o @ w_map.T + b_map

Sharding: 8 cores = (batch b in 0..3) x (context half h in 0..1).
Each core owns 1024 context rows of one batch. All compute is row-local
except the softmax-over-context (weight_two); its (sum-exp, weighted-sum)
stats are combined across the pair of cores sharing a batch via a tiny
pairwise AllReduce, overlapped with the large matmuls.

Math restructure (per stage, X = stage input [C,D], M = memory [Q,D]):
  out = X@W1 + o1@W2 + (X*o1)@W3 + (t*o1)@W4      (W_k = w_out[:, kD:(k+1)D].T)
  o1 = P@M (rank Q=64), t broadcast over rows =>
  o1@W2 + (t*o1)@W4 = P @ (M@W2 + (M*t)@W4)        (rank-64 path)

All big operands are bf16 (matmul rate on TRN2 is the same as f32r at
>=256 moving rows, but DMA/SBUF/DVE traffic halves); accumulation stays in
fp32 PSUM and softmax statistics are computed in fp32. The column-softmax
max/sum stats are obtained by PE-transposing the exp'd score tile and
reducing along the free axis on DVE (the gpsimd partition_all_reduce the
earlier version used was ~16.5us per call and serialized the stage).
"""

import numpy as np
import ml_dtypes

import concourse.bacc as bacc
import concourse.tile as tile
from concourse import mybir
from concourse.bass_utils import run_bass_kernel_spmd
from concourse.masks import make_identity
from contextlib import ExitStack
import bass_rust

f32 = mybir.dt.float32
bf16 = mybir.dt.bfloat16
i32 = mybir.dt.int32
Alu = mybir.AluOpType
AF = bass_rust.ActivationFunctionType
AX = bass_rust.AxisListType

B, C_LEN, Q_LEN, D = 4, 2048, 64, 1024
N_CORES = 8
R = C_LEN // 2          # rows per core
NK = D // 128           # contraction chunks
RH = R // 512           # row halves (moving-dim tiles)
NJ2 = 16                # output blocks of the final linear
D2 = 2 * D
NEGBIG = 10000.0
BF = ml_dtypes.bfloat16

_CACHED_NC = None


def _build_nc():
    nc = bacc.Bacc("TRN2", target_bir_lowering=False, debug=False,
                   num_devices=N_CORES)

    # ---- per-core DRAM I/O (host pre-tiled layouts, see _shard_inputs) ----
    xt_ap = nc.dram_tensor("xt", [128, NK * R], bf16, kind="ExternalInput").ap()
    m_t = [nc.dram_tensor(f"m{s}t", [128, NK * Q_LEN], bf16, kind="ExternalInput").ap() for s in (1, 2)]
    m_n = [nc.dram_tensor(f"m{s}n", [Q_LEN, D], bf16, kind="ExternalInput").ap() for s in (1, 2)]
    vec = [nc.dram_tensor(f"vec{s}", [128, NK * 3], f32, kind="ExternalInput").ap() for s in (1, 2)]
    wmb = [nc.dram_tensor(f"wmb{s}", [128, NK], bf16, kind="ExternalInput").ap() for s in (1, 2)]
    msk = [nc.dram_tensor(f"mask{s}", [Q_LEN, 1], i32, kind="ExternalInput").ap() for s in (1, 2)]
    w1t = [nc.dram_tensor(f"w1t{s}", [D, D], bf16, kind="ExternalInput").ap() for s in (1, 2)]
    w3t = [nc.dram_tensor(f"w3t{s}", [D, D], bf16, kind="ExternalInput").ap() for s in (1, 2)]
    w2c = [nc.dram_tensor(f"w2c{s}", [D, D], bf16, kind="ExternalInput").ap() for s in (1, 2)]
    w4c = [nc.dram_tensor(f"w4c{s}", [D, D], bf16, kind="ExternalInput").ap() for s in (1, 2)]
    wmt_ap = nc.dram_tensor("wmt", [D2, D], bf16, kind="ExternalInput").ap()
    bmap_ap = nc.dram_tensor("bmap", [128, NJ2], f32, kind="ExternalInput").ap()
    out_ap = nc.dram_tensor("out", [D2, R], bf16, kind="ExternalOutput").ap()

    with tile.TileContext(nc) as tc, ExitStack() as ctx:
        sb = ctx.enter_context(tc.tile_pool(name="sb", bufs=1))
        sb_xt = ctx.enter_context(tc.tile_pool(name="sb_xt", bufs=2))
        sb_xo = ctx.enter_context(tc.tile_pool(name="sb_xo", bufs=1))
        sb_w13 = ctx.enter_context(tc.tile_pool(name="sb_w13", bufs=16))
        sb_wmj = ctx.enter_context(tc.tile_pool(name="sb_wmj", bufs=16))
        sb_w24 = ctx.enter_context(tc.tile_pool(name="sb_w24", bufs=8))
        ps_sc = ctx.enter_context(tc.tile_pool(name="ps_sc", bufs=2, space="PSUM"))
        ps_tp = ctx.enter_context(tc.tile_pool(name="ps_tp", bufs=2, space="PSUM"))
        ps_bc = ctx.enter_context(tc.tile_pool(name="ps_bc", bufs=2, space="PSUM"))
        ps_jg = ctx.enter_context(tc.tile_pool(name="ps_jg", bufs=2, space="PSUM"))
        dram = ctx.enter_context(tc.tile_pool(name="dram", bufs=2, space="DRAM"))

        # ---- constants ----
        ident = sb.tile([128, 128], bf16, tag="ident")
        make_identity(nc, ident[:])
        ones_r = sb.tile([1, 128], bf16, tag="ones_r")
        nc.vector.memset(ones_r[:], 1.0)
        ones_c = sb.tile([128, 1], bf16, tag="ones_c")
        nc.vector.memset(ones_c[:], 1.0)

        # ---- stage input 1: X^T (software-DGE queue) ----
        xt0 = sb_xt.tile([128, NK, R], bf16, tag="xt")
        for c in range(NK):
            nc.gpsimd.dma_start(xt0[:, c], xt_ap[:, c * R:(c + 1) * R])

        # ---- per-stage constants + memory_dot / mbias, both stages up front
        st = [dict() for _ in range(2)]

        def prep_stage(s):
            d = st[s]
            vecs = sb.tile([128, NK, 3], f32, tag=f"vecs{s}")
            nc.scalar.dma_start(vecs[:], vec[s][:].rearrange("p (c k) -> p c k", c=NK))
            mT = sb.tile([128, NK, Q_LEN], bf16, tag=f"mT{s}")
            nc.scalar.dma_start(mT[:], m_t[s][:].rearrange("p (c q) -> p c q", c=NK))
            wmbs = sb.tile([128, NK], bf16, tag=f"wmb{s}")
            nc.scalar.dma_start(wmbs[:], wmb[s][:])
            mN = sb.tile([Q_LEN, D], bf16, tag=f"mN{s}")
            nc.scalar.dma_start(mN[:], m_n[s][:])
            mask_i = sb.tile([Q_LEN, 1], i32, tag=f"mask{s}")
            nc.scalar.dma_start(mask_i[:], msk[s][:])

            # mst = [M^T * scale | w_in]  (lhsT for the score matmul)
            mst = sb.tile([128, NK, Q_LEN + 1], bf16, tag=f"mst{s}")
            for c in range(NK):
                nc.vector.tensor_scalar(mst[:, c, 0:Q_LEN], mT[:, c],
                                        vecs[:, c, 2:3], None, Alu.mult)
            nc.vector.tensor_copy(mst[:, :, Q_LEN:Q_LEN + 1], vecs[:, :, 0:1])

            # memory_dot = M @ w_mem  -> psum [Q,1]
            ps_md = ps_tp.tile([Q_LEN, 1], f32, tag="tp")
            for c in range(NK):
                nc.tensor.matmul(ps_md[:], mT[:, c], wmbs[:, c:c + 1],
                                 start=(c == 0), stop=(c == NK - 1))
            maskf = sb.tile([Q_LEN, 1], f32, tag=f"maskf{s}")
            nc.vector.tensor_copy(maskf[:], mask_i[:])
            mbias = sb.tile([Q_LEN, 1], f32, tag=f"mbias{s}")
            nc.vector.tensor_scalar(mbias[:], maskf[:], NEGBIG, -NEGBIG,
                                    Alu.mult, Alu.add)
            nc.vector.tensor_tensor(mbias[:], mbias[:], ps_md[:], Alu.add)
            d.update(mT=mT, mN=mN, mst=mst, mbias=mbias)

        def load_w24(s):
            # software-DGE queue: keeps the Act hwdge queue free for compute
            w2ch, w4ch = [], []
            for c in range(NK):
                w2h = sb_w24.tile([128, 1024], bf16, tag="w2h")
                nc.gpsimd.dma_start(w2h[:], w2c[s][c * 128:(c + 1) * 128, :])
                w2ch.append(w2h)
            for c in range(NK):
                w4h = sb_w24.tile([128, 1024], bf16, tag="w4h")
                nc.gpsimd.dma_start(w4h[:], w4c[s][c * 128:(c + 1) * 128, :])
                w4ch.append(w4h)
            st[s].update(w2ch=w2ch, w4ch=w4ch)

        prep_stage(0)
        prep_stage(1)
        load_w24(0)

        bcols = sb.tile([128, NJ2], f32, tag="bcols")
        nc.scalar.dma_start(bcols[:], bmap_ap[:])

        def run_stage(s, Xt):
            """One BiAttention stage; returns o^T tile [128, NK, R] bf16."""
            d = st[s]
            mT, mN, mst, mbias = d["mT"], d["mN"], d["mst"], d["mbias"]
            w2ch, w4ch = d["w2ch"], d["w4ch"]

            w13 = {}

            def load_w13(j):
                w1j = sb_w13.tile([128, NK, 128], bf16, tag="w13")
                nc.sync.dma_start(w1j[:], w1t[s][j * 128:(j + 1) * 128, :]
                                  .rearrange("p (c m) -> p c m", c=NK))
                w3j = sb_w13.tile([128, NK, 128], bf16, tag="w13")
                nc.sync.dma_start(w3j[:], w3t[s][j * 128:(j + 1) * 128, :]
                                  .rearrange("p (c m) -> p c m", c=NK))
                w13[j] = (w1j, w3j)

            for j in range(NK):
                load_w13(j)

            P = sb.tile([Q_LEN, R], bf16, tag="P", bufs=2)
            st8 = sb.tile([128, RH, 8], bf16, tag="st8", bufs=2)
            vh = sb.tile([128, 2 * NK], f32, tag="vh", bufs=2)
            E_ = [None] * RH
            TP_ = [None] * RH
            rs_ = [None] * RH
            ebs_ = [None] * RH

            def scores(rh):
                sl = slice(rh * 512, (rh + 1) * 512)
                ps = ps_sc.tile([Q_LEN + 1, 512], f32, tag="sc")
                for c in range(NK):
                    nc.tensor.matmul(ps[:], mst[:, c], Xt[:, c, sl],
                                     start=(c == 0), stop=(c == NK - 1))
                E = sb.tile([Q_LEN + 1, 512], bf16, tag="E", bufs=2)
                nc.scalar.activation(E[0:Q_LEN], ps[0:Q_LEN], AF.Exp,
                                     bias=mbias[:], scale=1.0)
                nc.scalar.activation(E[Q_LEN:Q_LEN + 1], ps[Q_LEN:Q_LEN + 1],
                                     AF.Exp)
                E_[rh] = E

            def transposes(rh):
                # E chunks [65,128] -> [128,65] so q lands on the free axis
                TP = ps_tp.tile([128, 4, 80], bf16, tag="tp")
                for k in range(4):
                    nc.tensor.transpose(TP[:, k, 0:Q_LEN + 1],
                                        E_[rh][:, k * 128:(k + 1) * 128],
                                        ident[0:Q_LEN + 1, 0:Q_LEN + 1])
                TP_[rh] = TP

            def stats(rh):
                TP = TP_[rh]
                mx = sb.tile([128, 4], f32, tag="mx", bufs=2)
                nc.vector.reduce_max(mx[:], TP[:, :, 0:Q_LEN], AX.X)
                l1 = sb.tile([128, 4], f32, tag="l1", bufs=2)
                nc.vector.reduce_sum(l1[:], TP[:, :, 0:Q_LEN], AX.X)
                # e2 = (max_q E) * exp(input_dot); l1r = 1/l1
                nc.vector.tensor_tensor(st8[:, rh, 0:4], mx[:],
                                        TP[:, :, Q_LEN], Alu.mult)
                with nc.allow_low_precision(reason="softmax scales in bf16"):
                    nc.vector.reciprocal(st8[:, rh, 4:8], l1[:])

            def stats_rows(rh):
                # column stats [128,{e2|l1r}x4] -> row form [1, 1024] on p0
                tp8 = ps_tp.tile([1, 1024], bf16, tag="tp")
                for k in range(8):
                    nc.tensor.transpose(tp8[0:1, k * 128:(k + 1) * 128],
                                        st8[:, rh, k:k + 1], ident[:])
                rs = sb.tile([1, 1024], bf16, tag="rowst", bufs=2)
                nc.scalar.copy(rs[0:1, 0:512], tp8[0:1, 0:512])
                nc.vector.tensor_copy(rs[0:1, 512:1024], tp8[0:1, 512:1024])
                rs_[rh] = rs

            def bcasts(rh):
                rs = rs_[rh]
                eb = ps_bc.tile([128, 512], f32, tag="bc")
                for k in range(4):
                    nc.tensor.matmul(eb[:, k * 128:(k + 1) * 128], ones_r[:],
                                     rs[0:1, k * 128:(k + 1) * 128],
                                     start=True, stop=True)
                ebs = sb.tile([128, 512], bf16, tag="ebs", bufs=2)
                nc.scalar.copy(ebs[:], eb[:])
                ebs_[rh] = ebs
                rb = ps_bc.tile([Q_LEN, 512], f32, tag="bc")
                for k in range(4):
                    nc.tensor.matmul(rb[:, k * 128:(k + 1) * 128],
                                     ones_r[:, 0:Q_LEN],
                                     rs[0:1, (4 + k) * 128:(5 + k) * 128],
                                     start=True, stop=True)
                sl = slice(rh * 512, (rh + 1) * 512)
                nc.vector.tensor_tensor(P[:, sl], E_[rh][0:Q_LEN], rb[:],
                                        Alu.mult)

            # ---- PE schedule: scores/stats with pre-opened j=0 groups ----
            pre = []
            scores(0)
            for rh in range(RH):
                sl = slice(rh * 512, (rh + 1) * 512)
                pj = ps_jg.tile([128, 512], f32, tag="jg")
                for c in range(NK):
                    nc.tensor.matmul(pj[:], w13[0][0][:, c], Xt[:, c, sl],
                                     start=(c == 0), stop=False)
                pre.append(pj)
                if rh == 0:
                    transposes(0)
                    stats(0)
            scores(1)
            stats_rows(0)
            bcasts(0)
            transposes(1)
            stats(1)
            stats_rows(1)
            bcasts(1)

            # l2 = sum over all rows of e2  (both rh at once)
            ps_l2 = ps_tp.tile([1, RH * 4], f32, tag="tp")
            nc.tensor.matmul(ps_l2[:], ones_c[:], st8[:, :, 0:4],
                             start=True, stop=True)
            l2s = sb.tile([1, 1], f32, tag="l2s", bufs=2)
            nc.vector.reduce_sum(l2s[:], ps_l2[:], AX.X)

            # rank-64 path part 1: Rw2 accumulation opened early in the
            # freed score slots (PE filler during the stats chain)
            ps_r = []
            for hf in range(RH):
                slh = slice(hf * 512, (hf + 1) * 512)
                ps_ri = ps_sc.tile([Q_LEN, 512], f32, tag="sc")
                for c in range(NK):
                    nc.tensor.matmul(ps_ri[:], mT[:, c], w2ch[c][:, slh],
                                     start=(c == 0), stop=False)
                ps_r.append(ps_ri)

            # ---- o1^T = mN.T @ P and XO = Xt * o1, rh-major ----
            XO = sb_xo.tile([128, NK, R], bf16, tag="xo")
            oT = sb_xt.tile([128, NK, R], bf16, tag="xt")
            for rh in range(RH):
                sl = slice(rh * 512, (rh + 1) * 512)
                for c in range(NK):
                    ps_o1 = ps_bc.tile([128, 512], f32, tag="bc")
                    nc.tensor.matmul(ps_o1[:], mN[:, c * 128:(c + 1) * 128],
                                     P[:, sl], start=True, stop=True)
                    if c < 6:
                        nc.vector.tensor_tensor(XO[:, c, sl], Xt[:, c, sl],
                                                ps_o1[:], Alu.mult)
                    else:
                        # spread work: stage PSUM->SBUF via Act, multiply on Pool
                        o1s = sb.tile([128, 512], bf16, tag="o1s", bufs=2)
                        nc.scalar.copy(o1s[:], ps_o1[:])
                        nc.gpsimd.tensor_tensor(XO[:, c, sl], Xt[:, c, sl],
                                                o1s[:], Alu.mult)
                # close the pre-opened j=0 group with its W3 half
                pj = pre[rh]
                for c in range(NK):
                    nc.tensor.matmul(pj[:], w13[0][1][:, c], XO[:, c, sl],
                                     start=False, stop=(c == NK - 1))
                nc.scalar.copy(oT[:, 0, sl], pj[:])

            # v = sum over rows of e2*X  (accumulated on DVE+Pool from ebs)
            for rh in range(RH):
                sl = slice(rh * 512, (rh + 1) * 512)
                for c in range(NK):
                    scr = sb.tile([128, 512], bf16, tag="scrV", bufs=2)
                    nc.vector.scalar_tensor_tensor(
                        scr[:], Xt[:, c, sl], 1.0, ebs_[rh][:],
                        Alu.mult, Alu.mult,
                        accum_out=vh[:, 2 * c + rh:2 * c + rh + 1])

            # ---- pairwise AllReduce of (v, l2) ----
            vsum = sb.tile([128, NK], f32, tag="vsum", bufs=2)
            vh3 = vh[:].rearrange("p (c t) -> p c t", t=2)
            nc.vector.tensor_tensor(vsum[:], vh3[:, :, 0], vh3[:, :, 1], Alu.add)
            colsb = sb.tile([128, 16], f32, tag="colsb", bufs=2)
            nc.vector.memset(colsb[:], 0.0)
            nc.vector.tensor_copy(colsb[:, 0:NK], vsum[:])
            nc.vector.tensor_copy(colsb[0:1, NK:NK + 1], l2s[:])
            cin = dram.tile([128, 16], f32, tag="cin")
            cout = dram.tile([128, 16], f32, tag="cout")
            nc.sync.dma_start(cin[:], colsb[:])
            nc.gpsimd.collective_compute(
                "AllReduce", Alu.add,
                replica_groups=[[0, 1], [2, 3], [4, 5], [6, 7]],
                ins=[cin[:].opt()], outs=[cout[:].opt()])
            colg = sb.tile([128, 16], f32, tag="colg", bufs=2)
            nc.sync.dma_start(colg[:], cout[:])

            # ---- main W1/W3 blocks ----
            for j in range(1, NK):
                w1j, w3j = w13[j]
                for rh in range(RH):
                    sl = slice(rh * 512, (rh + 1) * 512)
                    ps_ab = ps_jg.tile([128, 512], f32, tag="jg")
                    for c in range(NK):
                        nc.tensor.matmul(ps_ab[:], w1j[:, c], Xt[:, c, sl],
                                         start=(c == 0), stop=False)
                    for c in range(NK):
                        nc.tensor.matmul(ps_ab[:], w3j[:, c], XO[:, c, sl],
                                         start=False, stop=(c == NK - 1))
                    nc.scalar.copy(oT[:, j, sl], ps_ab[:])

            # ---- collective-dependent tail ----
            linv = sb.tile([1, 1], bf16, tag="linv", bufs=2)
            with nc.allow_low_precision(reason="weight-two scale in bf16"):
                nc.vector.reciprocal(linv[:], colg[0:1, NK:NK + 1])
            ps_tb = ps_tp.tile([128, 1], f32, tag="tp")
            nc.tensor.matmul(ps_tb[:], ones_r[:], linv[:], start=True, stop=True)
            tvec = sb.tile([128, NK], f32, tag="tvec", bufs=2)
            nc.vector.tensor_scalar(tvec[:], colg[:, 0:NK], ps_tb[:, 0:1],
                                    None, Alu.mult)
            mTs = sb.tile([128, NK, Q_LEN], bf16, tag="mTs", bufs=2)
            for c in range(NK):
                nc.vector.tensor_scalar(mTs[:, c], mT[:, c], tvec[:, c:c + 1],
                                        None, Alu.mult)

            # rank-64 path part 2: += (M*t)@W4, then Rsb = psum
            Rsb = sb.tile([Q_LEN, D], bf16, tag="Rsb", bufs=2)
            for hf in range(RH):
                slh = slice(hf * 512, (hf + 1) * 512)
                for c in range(NK):
                    nc.tensor.matmul(ps_r[hf][:], mTs[:, c], w4ch[c][:, slh],
                                     start=False, stop=(c == NK - 1))
                nc.scalar.copy(Rsb[:, slh], ps_r[hf][:])

            # rank-64 correction (rh-major so the next stage starts sooner)
            for rh in range(RH):
                sl = slice(rh * 512, (rh + 1) * 512)
                for j in range(NK):
                    ps_c = ps_bc.tile([128, 512], f32, tag="bc")
                    nc.tensor.matmul(ps_c[:], Rsb[:, j * 128:(j + 1) * 128],
                                     P[:, sl], start=True, stop=True)
                    if j % 2 == 0:
                        nc.vector.tensor_tensor(oT[:, j, sl], oT[:, j, sl],
                                                ps_c[:], Alu.add)
                    else:
                        # Pool can't read PSUM: stage through SBUF via Act
                        cs = sb.tile([128, 512], bf16, tag="corrs", bufs=2)
                        nc.scalar.copy(cs[:], ps_c[:])
                        nc.gpsimd.tensor_tensor(oT[:, j, sl], oT[:, j, sl],
                                                cs[:], Alu.add)
            return oT

        o1T = run_stage(0, xt0)
        load_w24(1)
        # final-linear weights: all 16 blocks resident (software-DGE queue)
        wmjs = []
        for j2 in range(NJ2):
            wmj = sb_wmj.tile([128, NK, 128], bf16, tag="wmj")
            nc.gpsimd.dma_start(wmj[:], wmt_ap[j2 * 128:(j2 + 1) * 128, :]
                                .rearrange("p (c m) -> p c m", c=NK))
            wmjs.append(wmj)
        o2T = run_stage(1, o1T)

        # ---------- final linear (transposed): outT = w_mapT.T @ o2T + b ----
        for rh in range(RH):
            sl = slice(rh * 512, (rh + 1) * 512)
            for j2 in range(NJ2):
                ps_f = ps_jg.tile([128, 512], f32, tag="jg")
                for c in range(NK):
                    nc.tensor.matmul(ps_f[:], wmjs[j2][:, c], o2T[:, c, sl],
                                     start=(c == 0), stop=(c == NK - 1))
                outsb = sb.tile([128, 512], bf16, tag="outsb", bufs=4)
                if j2 % 2 == 0:
                    nc.scalar.activation(outsb[:], ps_f[:], AF.Identity,
                                         bias=bcols[:, j2:j2 + 1], scale=1.0)
                else:
                    nc.vector.tensor_scalar(outsb[:], ps_f[:],
                                            bcols[:, j2:j2 + 1], None, Alu.add)
                q = nc.sync if j2 % 2 == 0 else nc.scalar
                q.dma_start(out_ap[j2 * 128:(j2 + 1) * 128, sl], outsb[:])

    nc.compile()
    return nc


def _get_nc():
    global _CACHED_NC
    if _CACHED_NC is None:
        _CACHED_NC = _build_nc()
    return _CACHED_NC


def _shard_inputs(inputs):
    """Build the 8 per-core input maps (layout + dtype cast only)."""
    x = np.asarray(inputs["ctx_features"], dtype=np.float32)
    q1 = np.asarray(inputs["sub_q1_features"], dtype=np.float32)
    q2 = np.asarray(inputs["sub_q2_features"], dtype=np.float32)
    k1 = np.ascontiguousarray(np.asarray(inputs["sub_q1_attn_mask"], dtype=np.int32))
    k2 = np.ascontiguousarray(np.asarray(inputs["sub_q2_attn_mask"], dtype=np.int32))

    def wblocks(w_out):
        # w_out [D, 4D] -> wb = w_out.T [4D, D]; W_k = wb[kD:(k+1)D]
        wb = np.ascontiguousarray(w_out.T.astype(BF))
        W1, W2, W3, W4 = (wb[k * D:(k + 1) * D] for k in range(4))

        def jmaj(W):  # j-major tiling for the lhsT stream
            return np.ascontiguousarray(
                W.reshape(NK, 128, NK, 128).transpose(2, 1, 0, 3).reshape(D, D))
        return jmaj(W1), np.ascontiguousarray(W2), jmaj(W3), np.ascontiguousarray(W4)

    w1t1, w2c1, w3t1, w4c1 = wblocks(inputs["w_out1"])
    w1t2, w2c2, w3t2, w4c2 = wblocks(inputs["w_out2"])

    wmT = inputs["w_map"].T.astype(BF)  # [D, 2D]
    wmt = np.ascontiguousarray(
        wmT.reshape(NK, 128, NJ2, 128).transpose(2, 1, 0, 3).reshape(D2, D))
    bmap = np.ascontiguousarray(
        np.asarray(inputs["b_map"], dtype=np.float32).reshape(NJ2, 128).T)

    def ptile_vec(*cols):  # [D] vectors -> [128, NK*k] p-major
        v = np.stack([np.asarray(c, dtype=np.float32) for c in cols], axis=-1)
        k = v.shape[-1]
        return np.ascontiguousarray(
            v.reshape(NK, 128, k).transpose(1, 0, 2).reshape(128, NK * k))

    def pmaj(vecd, dt=BF):  # [D] -> [128, NK] p-major
        return np.ascontiguousarray(
            np.asarray(vecd).reshape(NK, 128).T.astype(dt))

    stage_common = {
        "vec1": ptile_vec(inputs["w_in1"], inputs["w_mem1"], inputs["scale1"]),
        "vec2": ptile_vec(inputs["w_in2"], inputs["w_mem2"], inputs["scale2"]),
        "wmb1": pmaj(inputs["w_mem1"]),
        "wmb2": pmaj(inputs["w_mem2"]),
        "w1t1": w1t1, "w3t1": w3t1, "w2c1": w2c1, "w4c1": w4c1,
        "w1t2": w1t2, "w3t2": w3t2, "w2c2": w2c2, "w4c2": w4c2,
        "wmt": wmt, "bmap": bmap,
    }

    in_maps = []
    for core in range(N_CORES):
        b, h = divmod(core, 2)
        xT = x[b, h * R:(h + 1) * R, :].T.astype(BF)  # [D, R]
        xt_tile = np.ascontiguousarray(
            xT.reshape(NK, 128, R).transpose(1, 0, 2).reshape(128, NK * R))
        m = {}
        for s, q, kk in ((1, q1, k1), (2, q2, k2)):
            mT = q[b].T.astype(BF)  # [D, Q]
            m[f"m{s}t"] = np.ascontiguousarray(
                mT.reshape(NK, 128, Q_LEN).transpose(1, 0, 2).reshape(128, NK * Q_LEN))
            m[f"m{s}n"] = np.ascontiguousarray(q[b].astype(BF))
            m[f"mask{s}"] = np.ascontiguousarray(kk[b].reshape(Q_LEN, 1))
        in_maps.append({"xt": xt_tile, **m, **stage_common})
    return in_maps


def _gather_outputs(results):
    out = np.empty((B, C_LEN, D2), dtype=np.float32)
    for core in range(N_CORES):
        b, h = divmod(core, 2)
        out[b, h * R:(h + 1) * R, :] = results[core]["out"].astype(np.float32).T
    return out


def kernel(**inputs):
    nc = _get_nc()
    in_maps = _shard_inputs(inputs)
    last_err = None
    for _attempt in range(3):
        try:
            res = run_bass_kernel_spmd(nc, in_maps, core_ids=list(range(N_CORES)))
            return _gather_outputs(res.results)
        except Exception as e:  # transient device errors: retry
            last_err = e
    raise last_err
